# revision 26
# baseline (speedup 1.0000x reference)
"""Trainium2 Bass kernel for a multi-head-attention block (B,C,N,D = 8,4,1024,96;
H=3 heads, dk=dv=32; softmax over the QUERY axis; residual + LayerNorm).

Sharding: pure data-parallel over batch B across 8 NeuronCores (one batch
element per core, C=4 channel-slices each, no collectives).

Schedule notes (v2):
  - exp work is split across engines: per chunk, heads 0/1 exp on ACT
    (spline exp + fused accumulator denominator + accumulator read), and
    head 2 of DVE-light slots runs a Schraudolph bit-hack exp on the DVE
    (one tensor_scalar: int16 = round(scores*A + B), bitcast bf16) with a
    deferred tensor_reduce denominator.  Per-tile constant factors cancel
    exactly in the q-axis softmax, so the ~4% pointwise exp error averages
    to ~1e-3 relative in the final output.
  - softmax normalization (vsc = V/denom, plus 1/denom write-back nobody
    reads) runs on the otherwise-idle GPSIMD via normalize_recip (attn
    ucode library), freeing the DVE.
  - PSUM pools are split so the scores ring (w_psum, 2 bufs x 2 banks)
    holds ONLY score tiles: the ring dependency is exp(i,h) -> scores MM,
    an ACT->PE edge with no DVE cast interposed.  Filler tiles
    (transpose/projection/v/fc) live in their own 2x1-bank pool (fp) and
    self-pace against their casts.  ctx accumulator: 2 banks.  8 total.
  - inputs arrive in DRAM twice: f32 xq (residual path) and host-rounded
    bf16 xq/xk/xv (projection path).  PE transposes are is_transpose
    matmuls with a bf16 identity writing bf16 PSUM (half the banks, one
    2x-mode cast per input instead of two 1x f32 casts).
  - lag-2 chunk pipeline: norm_recip(c,i-1) behind scores/exp(c,i), and
    ctx(c,i-2) one slot later so the PE never waits the gpsimd round trip;
    channel boundaries keep the same cadence.
  - LN tail: residual via scalar_tensor_tensor, bn_stats/bn_aggr for
    mean/var, rsqrt via bit-hack + 2 Newton iterations (all DVE);
    channel-3 tail is pipelined per q-half with ACT-based ln_out.
"""

from contextlib import ExitStack

import ml_dtypes
import numpy as np

import concourse.bass as bass
import concourse.tile as tile
from concourse import bacc, mybir
from concourse.bass_utils import run_bass_kernel_spmd

F32 = mybir.dt.float32
BF16 = mybir.dt.bfloat16
F32R = mybir.dt.float32r
I32 = mybir.dt.int32
I16 = mybir.dt.int16
A = mybir.AluOpType

B, C, N, D = 8, 4, 1024, 96
H, DK, DV = 3, 32, 32
P = 128               # partition size / token chunk
NCHUNK = N // P       # 8
QT = 512              # matmul free-dim limit into one PSUM bank (f32)
HN = N // 2
SCALE = 1.0 / np.sqrt(DK)
EPS = 1e-5

# Schraudolph exp on the DVE: bits(bf16 e) = round(x*SCALE*A + B) as int16.
SCH_A = float((2.0 ** 7) / np.log(2.0) * SCALE)
SCH_B = float(127 * 2 ** 7)


def _dve_tile(c, i):
    """Chunks whose head-2 exp runs on the DVE instead of ACT: the
    engine-balance knob, placed in DVE-light slots only.  The final chunk
    stays on ACT so the epilogue's normalization chain starts sooner."""
    if c == C - 1 and i == NCHUNK - 1:
        return False
    return (c >= 1 and i in (0, 1, 4, 5, 7)) or (c == 0 and i in (1, 7))

_CACHE = {}


def _emit(nc, tc, ctx, apply_affine):
    xq_d = nc.dram_tensor("xq", [C, N, D], F32R, kind="ExternalInput").ap()
    xqb_d = nc.dram_tensor("xqb", [C, N, D], BF16, kind="ExternalInput").ap()
    xkb_d = nc.dram_tensor("xkb", [C, N, D], BF16, kind="ExternalInput").ap()
    xvb_d = nc.dram_tensor("xvb", [C, N, D], BF16, kind="ExternalInput").ap()
    # wall  = host-packed f32 [128, 96+2]: wfc.T (zero-padded) | gamma | beta
    # wallb = host-packed bf16 [128, 128 + 3*96]: identity | wq|wk|wv (each
    #         W.T, [96,96] natural, zero-padded to 128 rows)
    wall_d = nc.dram_tensor("wall", [P, D + 2], F32R,
                            kind="ExternalInput").ap()
    wallb_d = nc.dram_tensor("wallb", [P, P + 3 * D], BF16,
                             kind="ExternalInput").ap()
    out_d = nc.dram_tensor("out", [C, N, D], F32, kind="ExternalOutput").ap()

    const = ctx.enter_context(tc.tile_pool(name="const", bufs=1))
    pc = ctx.enter_context(tc.tile_pool(name="perc", bufs=2))
    w_psum = ctx.enter_context(tc.tile_pool(name="w_psum", bufs=3, space="PSUM"))
    ctx_psum = ctx.enter_context(tc.tile_pool(name="ctx_psum", bufs=1, space="PSUM"))

    # ---- w_psum ring discipline: pad allocations to multiples of 3 so the
    # 3 scores tiles of chunk i+1 land exactly on the banks freed by the
    # 3 exps of chunk i (same head -> earliest possible reuse).
    wct = {"n": 0, "pad": 0}

    def wtile(shape, dtype, name):
        wct["n"] += 1
        return w_psum.tile(shape, dtype, name=name, tag="w")

    def wpad():
        while wct["n"] % 3:
            wct["n"] += 1
            wct["pad"] += 1
            w_psum.tile([P, 8], F32, name=f"pad{wct['pad']}", tag="w")

    # ---------------- prologue: DMAs + PE warm-up spins ----------------
    dummy = const.tile([P, QT], BF16)
    nc.vector.memset(dummy, 0)

    # identity (gates the first transposes) before the weight columns
    wallb = const.tile([P, P + 3 * D], BF16)
    nc.sync.dma_start(out=wallb[:, 0:P], in_=wallb_d[:, 0:P])
    nc.sync.dma_start(out=wallb[:, P:], in_=wallb_d[:, P:])
    ident_b = wallb[:, 0:P]
    wb = {nm: wallb[0:D, P + k * D:P + (k + 1) * D]
          for k, nm in enumerate(("wq", "wk", "wv"))}

    # f32 side: wfc (first needed at fc_group(0) in slot (1,2)) + ln affine
    wall = const.tile([P, D + 2], F32R)
    nc.gpsimd.dma_start(out=wall, in_=wall_d)
    wfc = wall[0:D, 0:D]

    # channel-0 loads: bf16 projections path split for earliest q
    xq0 = pc.tile([P, NCHUNK, D], F32R, name="xq_nat0", tag="xq_nat", bufs=3)
    xqb0 = pc.tile([P, NCHUNK, D], BF16, name="xqb0", tag="xqb", bufs=1)
    xkb0 = pc.tile([P, NCHUNK, D], BF16, name="xkb0", tag="xkb", bufs=1)
    xvb0 = pc.tile([P, NCHUNK, D], BF16, name="xvb0", tag="xvb", bufs=1)
    xqb0_r = xqb_d[0].rearrange("(i p) d -> p i d", p=P)
    xkb0_r = xkb_d[0].rearrange("(i p) d -> p i d", p=P)
    # ACT's DMA queue is free before the first exp; one DMA per queue to
    # avoid per-queue DGE serialization
    nc.scalar.dma_start(out=xqb0[:, 0:4, :], in_=xqb0_r[:, 0:4, :])
    nc.gpsimd.dma_start(out=xqb0[:, 4:8, :], in_=xqb0_r[:, 4:8, :])
    nc.gpsimd.dma_start(out=xkb0[:, 0:4, :], in_=xkb0_r[:, 0:4, :])
    nc.gpsimd.dma_start(out=xkb0[:, 4:8, :], in_=xkb0_r[:, 4:8, :])
    nc.gpsimd.dma_start(out=xvb0, in_=xvb_d[0].rearrange("(i p) d -> p i d", p=P))
    nc.sync.dma_start(out=xq0, in_=xq_d[0].rearrange("(i p) d -> p i d", p=P))

    spin = wtile([P, N], F32, "spin")
    for _ in range(2):
        nc.tensor.matmul(spin[0:64, 0:256], lhsT=dummy[:, 0:64],
                         rhs=dummy[:, 0:256], start=True, stop=True,
                         skip_group_check=True)

    # warm up the gpsimd attn ucode library (normalize_recip): the inserted
    # MODIFY_POOL_CONFIG + ~6us IRAM load runs now, hidden under channel-0
    # compute, instead of stalling the first real norm_recip mid-pipeline
    warm = const.tile([P, 4], F32)
    nc.vector.memset(warm, 1.0)
    nc.gpsimd.normalize_recip(out_ap=warm[:, 2:3], in_ap=warm[:, 0:1],
                              denom_ap=warm[:, 1:2])

    gam_tile = bet_tile = None
    if apply_affine:
        gam_tile = const.tile([P, D], F32)
        bet_tile = const.tile([P, D], F32)
        for t, col in ((gam_tile, D), (bet_tile, D + 1)):
            col_ap = wall_d[0:D, col:col + 1]
            bcast = bass.AP(tensor=col_ap.tensor, offset=col_ap.offset,
                            ap=[[0, P], col_ap.ap[0]])
            nc.gpsimd.dma_start(out=t, in_=bcast)

    st = {0: dict(xq_nat=xq0, xqb=xqb0, xkb=xkb0, xvb=xvb0, xTs={})}
    ts = {}

    def alloc_chunk_state(c):
        s = st[c]
        s["ssum"] = pc.tile([P, H * NCHUNK], F32, name=f"ssum{c}", tag="ssum")
        s["e_all"] = pc.tile([P, H * NCHUNK, N], BF16, name=f"e{c}", tag="e")
        s["vsc_all"] = pc.tile([P, H * NCHUNK, DV], BF16, name=f"vsc{c}",
                               tag="vsc")

    def loads(c):
        """DMA loads for channel c (c >= 1): no triggers on the Scalar eng."""
        xq = pc.tile([P, NCHUNK, D], F32R, name=f"xq_nat{c}", tag="xq_nat",
                     bufs=3)
        xqb = pc.tile([P, NCHUNK, D], BF16, name=f"xqb{c}", tag="xqb", bufs=1)
        xkb = pc.tile([P, NCHUNK, D], BF16, name=f"xkb{c}", tag="xkb", bufs=1)
        xvb = pc.tile([P, NCHUNK, D], BF16, name=f"xvb{c}", tag="xvb", bufs=1)
        nc.sync.dma_start(out=xq, in_=xq_d[c].rearrange("(i p) d -> p i d", p=P))
        nc.sync.dma_start(out=xqb, in_=xqb_d[c].rearrange("(i p) d -> p i d", p=P))
        nc.gpsimd.dma_start(out=xkb, in_=xkb_d[c].rearrange("(i p) d -> p i d", p=P))
        nc.gpsimd.dma_start(out=xvb, in_=xvb_d[c].rearrange("(i p) d -> p i d", p=P))
        st[c] = dict(xq_nat=xq, xqb=xqb, xkb=xkb, xvb=xvb, xTs={})

    def tr(c, nm, g=None):
        """PE is_transpose (bf16 in/out PSUM) of input nm; g=None does all
        8 chunks with a single 2x-mode cast, g in {0,1} does half."""
        s = st[c]
        src = s[f"x{nm}b"]
        if nm not in s["xTs"]:
            s["xTs"][nm] = pc.tile([D, N], BF16, name=f"x{nm}T{c}",
                                   tag=f"x{nm}T", bufs=1)
        xT = s["xTs"][nm]
        if g is None:
            tp = wtile([D, N], BF16, f"tp{nm}{c}")
            for i in range(NCHUNK):
                nc.tensor.transpose(tp[:, i * P:(i + 1) * P],
                                    in_=src[:, i, :], identity=ident_b)
            nc.vector.tensor_copy(out=xT, in_=tp)
        else:
            tp = wtile([D, HN], BF16, f"tp{nm}{c}{g}")
            for j in range(4):
                i = 4 * g + j
                nc.tensor.transpose(tp[:, j * P:(j + 1) * P],
                                    in_=src[:, i, :], identity=ident_b)
            nc.vector.tensor_copy(out=xT[:, g * HN:(g + 1) * HN], in_=tp)

    def proj_qk(c, which, g, on_act=False):
        """Q or K projection into [e, tok] bf16 layout, qtile g.  on_act
        moves the PSUM->SBUF cast to the (idle) Scalar engine -- prologue
        only, to parallelize the cast chain to the first exp."""
        s = st[c]
        nm, w_t = (("qdT", wb["wq"]) if which == "q" else ("kdT", wb["wk"]))
        if nm not in s:
            s[nm] = pc.tile([D, N], BF16, name=f"{nm}{c}", tag=nm)
        dst, xT = s[nm], s["xTs"][which]
        pr_ps = wtile([D, QT], F32, f"pr{c}{which}{g}")
        nc.tensor.matmul(pr_ps, lhsT=w_t, rhs=xT[:, g * QT:(g + 1) * QT],
                         start=True, stop=True)
        if on_act:
            nc.scalar.copy(out=dst[:, g * QT:(g + 1) * QT], in_=pr_ps)
        else:
            nc.vector.tensor_copy(out=dst[:, g * QT:(g + 1) * QT], in_=pr_ps)

    def v_g(c, g):
        """V projection (natural f32 layout), chunks 4g..4g+3"""
        s = st[c]
        if "v_nat" not in s:
            s["v_nat"] = pc.tile([P, NCHUNK, D], F32, name=f"v_nat{c}",
                                 tag="v_nat")
        v_nat = s["v_nat"]
        v_ps = wtile([P, 4 * D], F32, f"vps{c}{g}")
        for j in range(4):
            i = 4 * g + j
            nc.tensor.matmul(v_ps[:, j * D:(j + 1) * D],
                             lhsT=s["xTs"]["v"][:, i * P:(i + 1) * P],
                             rhs=wb["wv"], start=True, stop=True)
        nc.vector.tensor_copy(
            out=v_nat[:, 4 * g:4 * (g + 1), :].rearrange("p i d -> p (i d)"),
            in_=v_ps)

    pending_reduce = []

    def exp_tile(c, i, h):
        """exp for one (chunk, head) score tile.  ACT: spline exp + fused
        accumulator.  DVE: Schraudolph tensor_scalar; its tensor_reduce
        denominator is deferred to the slot end (a full slot of slack
        before the gpsimd norm_recip needs it)."""
        s = st[c]
        j = i * H + h
        if h == 2 and _dve_tile(c, i):
            nc.vector.tensor_scalar(
                out=s["e_all"][:, j, :].bitcast(I16), in0=s["s_regs"][h],
                scalar1=SCH_A, scalar2=SCH_B, op0=A.mult, op1=A.add)
            pending_reduce.append((c, j))
        else:
            nc.scalar.activation(
                out=s["e_all"][:, j, :], in_=s["s_regs"][h],
                func=mybir.ActivationFunctionType.Exp,
                scale=SCALE, accum_out=s["ssum"][:, j:j + 1])

    def flush_reduce():
        while pending_reduce:
            c_, j_ = pending_reduce.pop(0)
            s_ = st[c_]
            nc.vector.tensor_reduce(
                out=s_["ssum"][:, j_:j_ + 1], in_=s_["e_all"][:, j_, :],
                axis=mybir.AxisListType.X, op=A.add)

    def scores_exp(c, i):
        """S_T + exp for chunk i.  The three heads' matmuls are adjacent at
        row-groups 0/32/64 so they run concurrently in the PE array."""
        s = st[c]
        s_regs = [wtile([P, N], F32, f"s{c}_{i}_{h}") for h in range(H)]
        s["s_regs"] = s_regs
        for g in range(2):
            for h in range(H):
                hs = slice(DK * h, DK * (h + 1))
                nc.tensor.matmul(
                    s_regs[h][:, g * QT:(g + 1) * QT],
                    lhsT=s["kdT"][hs, i * P:(i + 1) * P],
                    rhs=s["qdT"][hs, g * QT:(g + 1) * QT],
                    start=True, stop=True)
        for h in range(H):
            exp_tile(c, i, h)

    def vsc_one(c, i, h):
        s = st[c]
        hs = slice(DV * h, DV * (h + 1))
        j = i * H + h
        nc.gpsimd.normalize_recip(
            out_ap=s["vsc_all"][:, j, :], in_ap=s["v_nat"][:, i, hs],
            denom_ap=s["ssum"][:, j:j + 1])

    def vsc_mm(c, i, heads=range(H)):
        """fold 1/denom into V rows for chunk i: gpsimd normalize_recip
        (vsc = v/denom; ssum overwritten with 1/denom, unread)."""
        for h in heads:
            vsc_one(c, i, h)

    def ctx_mm(c, i):
        """context accumulation for chunk i: bf16, three heads at
        col-groups 0/32/64, emitted adjacently -> concurrent."""
        s = st[c]
        for g in range(2):
            for h in range(H):
                hs = slice(DV * h, DV * (h + 1))
                j = i * H + h
                nc.tensor.matmul(
                    s["ctx_ps"][hs, g * QT:(g + 1) * QT],
                    lhsT=s["vsc_all"][:, j, :],
                    rhs=s["e_all"][:, j, g * QT:(g + 1) * QT],
                    start=(i == 0), stop=(i == NCHUNK - 1),
                    skip_group_check=True)

    def tail_a(c, half=None):
        """ctx copy-out (frees the ctx PSUM banks); half in {0,1,None}"""
        s = st[c]
        if "ctxT" not in s:
            s["ctxT"] = pc.tile([D, N], F32R, name=f"ctxT{c}", tag="ctxT")
        halves = (0, 1) if half is None else (half,)
        for g in halves:
            nc.vector.tensor_copy(out=s["ctxT"][:, g * QT:(g + 1) * QT],
                                  in_=s["ctx_ps"][:, g * QT:(g + 1) * QT])
        if half in (1, None):
            del s["ctx_ps"]

    def fc_group(c, g):
        """fc matmuls + residual add + one bn_stats pass for chunks 4g..4g+3"""
        s = st[c]
        if c not in ts:
            ts[c] = dict(
                t_all=pc.tile([P, NCHUNK, D], F32, name=f"tall{c}", tag="tall"),
                bst=pc.tile([P, NCHUNK, 6], F32, name=f"bst{c}", tag="bst"),
                mv=pc.tile([P, NCHUNK, 2], F32, name=f"mv{c}", tag="mv"),
            )
        t = ts[c]
        fc_ps = wtile([P, 4 * D], F32, f"fc{c}{g}")
        for j in range(4):
            i = 4 * g + j
            nc.tensor.matmul(fc_ps[:, j * D:(j + 1) * D],
                             lhsT=s["ctxT"][:, i * P:(i + 1) * P],
                             rhs=wfc, start=True, stop=True)
        for j in range(4):
            i = 4 * g + j
            nc.vector.scalar_tensor_tensor(
                out=t["t_all"][:, i, :], in0=fc_ps[:, j * D:(j + 1) * D],
                scalar=1.0, in1=s["xq_nat"][:, i, :].bitcast(F32),
                op0=A.mult, op1=A.add)
            nc.vector.bn_stats(out=t["bst"][:, i, :], in_=t["t_all"][:, i, :])

    def ln_stats(c, lo=0, hi=NCHUNK):
        """per-chunk bn_aggr -> (mean, var); rstd via bit-hack + 2 Newton
        iterations (DVE)"""
        t = ts[c]
        if "var" not in t:
            t["var"] = pc.tile([P, NCHUNK], F32, name=f"var{c}", tag="var")
            t["y"] = pc.tile([P, NCHUNK], F32, name=f"y{c}", tag="y")
            t["t1"] = pc.tile([P, NCHUNK], F32, name=f"t1n{c}", tag="t1n")
        sl = slice(lo, hi)
        var, y, t1 = t["var"][:, sl], t["y"][:, sl], t["t1"][:, sl]
        for i in range(lo, hi):
            nc.vector.bn_aggr(out=t["mv"][:, i, :], in_=t["bst"][:, i, :])
        nc.vector.tensor_scalar_add(
            out=var, in0=t["mv"][:, sl, 1:2].rearrange("p i o -> p (i o)"),
            scalar1=EPS)
        nc.vector.tensor_scalar(
            out=y.bitcast(I32), in0=var.bitcast(I32), scalar1=1,
            scalar2=None, op0=A.logical_shift_right)
        nc.vector.tensor_scalar(
            out=y.bitcast(I32), in0=y.bitcast(I32), scalar1=-1,
            scalar2=None, op0=A.bitwise_xor)
        nc.vector.tensor_scalar(
            out=y.bitcast(I32), in0=y.bitcast(I32), scalar1=0x5F3759E0,
            scalar2=None, op0=A.add)
        for _ in range(2):
            nc.vector.tensor_mul(out=t1, in0=y, in1=y)
            nc.vector.tensor_mul(out=t1, in0=t1, in1=var)
            nc.vector.tensor_scalar(out=t1, in0=t1, scalar1=-0.5, scalar2=1.5,
                                    op0=A.mult, op1=A.add)
            nc.vector.tensor_mul(out=y, in0=y, in1=t1)

    def ln_out(c, lo, hi, on_act=False):
        """(t - mean) * rstd, chunks lo..hi-1.  on_act runs it on the
        Scalar engine as t*rstd + (-mean*rstd) - only useful in the
        epilogue when the exps are finished and ACT is idle."""
        t = ts[c]
        if "out_sb" not in t:
            t["out_sb"] = pc.tile([P, NCHUNK, D], F32, name=f"osb{c}",
                                  tag="osb")
        if on_act:
            if "nmb" not in t:
                t["nmb"] = pc.tile([P, NCHUNK], F32, name=f"nmb{c}",
                                   tag="nmb")
            nc.vector.scalar_tensor_tensor(
                out=t["nmb"][:, lo:hi],
                in0=t["mv"][:, lo:hi, 0:1].rearrange("p i o -> p (i o)"),
                scalar=-1.0, in1=t["y"][:, lo:hi], op0=A.mult, op1=A.mult)
            for i in range(lo, hi):
                nc.scalar.activation(
                    out=t["out_sb"][:, i, :], in_=t["t_all"][:, i, :],
                    func=mybir.ActivationFunctionType.Identity,
                    scale=t["y"][:, i:i + 1], bias=t["nmb"][:, i:i + 1])
            return
        for i in range(lo, hi):
            nc.vector.tensor_scalar(
                out=t["out_sb"][:, i, :], in0=t["t_all"][:, i, :],
                scalar1=t["mv"][:, i, 0:1], scalar2=t["y"][:, i:i + 1],
                op0=A.subtract, op1=A.mult)
        if apply_affine:
            for i in range(lo, hi):
                nc.vector.tensor_mul(out=t["out_sb"][:, i, :],
                                     in0=t["out_sb"][:, i, :], in1=gam_tile)
                nc.vector.tensor_add(out=t["out_sb"][:, i, :],
                                     in0=t["out_sb"][:, i, :], in1=bet_tile)

    def store(c, half=None):
        t = ts[c]
        out_r = out_d[c].rearrange("(i p) d -> p i d", p=P)
        if half is None:
            nc.sync.dma_start(out=out_r, in_=t["out_sb"])
        elif half == 0:
            nc.sync.dma_start(out=out_r[:, 0:4, :], in_=t["out_sb"][:, 0:4, :])
        else:
            nc.sync.dma_start(out=out_r[:, 4:8, :], in_=t["out_sb"][:, 4:8, :])

    # ------------- channel-0 minimal critical path to first exp -------------
    alloc_chunk_state(0)
    tr(0, "q", 0)
    tr(0, "q", 1)
    proj_qk(0, "q", 0, on_act=True)
    proj_qk(0, "q", 1)
    tr(0, "k", 0)
    proj_qk(0, "k", 0, on_act=True)
    wpad()
    scores_exp(0, 0)

    # per-slot fillers; emitted AFTER the slot's scores+ctx matmuls.
    def fillers(c, i):
        if c == 0:
            if i == 0:
                tr(0, "k", 1)
                proj_qk(0, "k", 1)
                tr(0, "v", 0)
                v_g(0, 0)
            elif i == 1:
                tr(0, "v", 1)
                v_g(0, 1)
                loads(1)
            elif i == 2:
                tr(1, "q")
            elif i == 3:
                tr(1, "k")
            elif i == 4:
                tr(1, "v")
            elif i == 5:
                proj_qk(1, "q", 0)
            elif i == 6:
                proj_qk(1, "q", 1)
                proj_qk(1, "k", 0)
                proj_qk(1, "k", 1)
            # i == 7 left empty so scores(1, 0) is never gated on a filler
            # cast stuck behind slot-7 DVE work
        else:
            # lag-2 ctx pipeline: ctx(c-1, 7) lands in slot (c, 1), so the
            # previous channel's tail shifts one slot later
            if i == 0:
                v_g(c, 0)
                v_g(c, 1)
                if c + 1 < C:
                    loads(c + 1)
            elif i == 1:
                tail_a(c - 1)
            elif i == 2:
                if c + 1 < C:
                    tr(c + 1, "q")
                fc_group(c - 1, 0)
            elif i == 3:
                if c + 1 < C:
                    tr(c + 1, "k")
                fc_group(c - 1, 1)
            elif i == 4:
                if c + 1 < C:
                    tr(c + 1, "v")
                ln_stats(c - 1)
            elif i == 5:
                if c + 1 < C:
                    proj_qk(c + 1, "q", 0)
                ln_out(c - 1, 0, 4)
                store(c - 1, half=0)
            elif i == 6:
                if c + 1 < C:
                    proj_qk(c + 1, "q", 1)
                    proj_qk(c + 1, "k", 0)
                    proj_qk(c + 1, "k", 1)
                ln_out(c - 1, 4, NCHUNK)
                store(c - 1, half=1)
            # i == 7 left empty (see channel-0 note)

    # ---------------- software-pipelined channel loop ----------------
    # lag-2 chunk pipeline (see docstring)
    for c in range(C):
        if c > 0:
            alloc_chunk_state(c)
        st[c]["ctx_ps"] = ctx_psum.tile([D, N], F32, name=f"ctx{c}", tag="ctx")
        for i in range(NCHUNK):
            if not (c == 0 and i == 0):
                scores_exp(c, i)
            pc_, pi = (c, i - 1) if i >= 1 else (c - 1, NCHUNK - 1)
            if pc_ >= 0:
                vsc_mm(pc_, pi, heads=(0, 1))
            if i >= 2:
                ctx_mm(c, i - 2)
            elif c >= 1 and i == 0:
                ctx_mm(c - 1, NCHUNK - 2)
            elif c >= 1 and i == 1:
                ctx_mm(c - 1, NCHUNK - 1)
            fillers(c, i)
            # deferred Schraudolph denominator at slot end (no downstream
            # pressure: its norm_recip feeds ctx a full slot later), then
            # the head-2 normalization of the previous chunk
            flush_reduce()
            if pc_ >= 0:
                vsc_mm(pc_, pi, heads=(2,))
            wpad()

    # ---------------- pipelined epilogue for channel 3 ----------------
    c = C - 1
    s, li = st[c], NCHUNK - 1
    flush_reduce()
    ctx_mm(c, NCHUNK - 2)
    for h in range(H):
        j = li * H + h
        hs = slice(DV * h, DV * (h + 1))
        nc.gpsimd.normalize_recip(
            out_ap=s["vsc_all"][:, j, :], in_ap=s["v_nat"][:, li, hs],
            denom_ap=s["ssum"][:, j:j + 1])
        for g in range(2):
            nc.tensor.matmul(
                s["ctx_ps"][hs, g * QT:(g + 1) * QT],
                lhsT=s["vsc_all"][:, j, :],
                rhs=s["e_all"][:, j, g * QT:(g + 1) * QT],
                start=False, stop=True, skip_group_check=True)
    tail_a(c, half=0)
    fc_group(c, 0)
    tail_a(c, half=1)
    ln_stats(c, 0, 4)
    fc_group(c, 1)
    out_r = out_d[c].rearrange("(i p) d -> p i d", p=P)
    t3 = None
    for lo in range(0, NCHUNK, 2):
        if lo == 4:
            ln_stats(c, 4, NCHUNK)
        ln_out(c, lo, lo + 2, on_act=True)
        t3 = ts[c]
        nc.sync.dma_start(out=out_r[:, lo:lo + 2, :],
                          in_=t3["out_sb"][:, lo:lo + 2, :])


def _build(apply_affine):
    nc = bacc.Bacc("TRN2", target_bir_lowering=False, debug=False, num_devices=B)
    with tile.TileContext(nc) as tc, ExitStack() as ctx:
        _emit(nc, tc, ctx, apply_affine)
    nc.compile()
    return nc


def _pack_in_maps(input_Q, input_K, input_V, W_Q, W_K, W_V, W_fc,
                  ln_gamma, ln_beta):
    bf = ml_dtypes.bfloat16
    wall = np.zeros((P, D + 2), dtype=np.float32)
    wall[0:D, 0:D] = W_fc.T
    wall[0:D, D] = ln_gamma
    wall[0:D, D + 1] = ln_beta
    wallb = np.zeros((P, P + 3 * D), dtype=bf)
    wallb[:, :P] = np.eye(P, dtype=np.float32).astype(bf)
    for k, W in enumerate((W_Q, W_K, W_V)):
        wallb[0:D, P + k * D:P + (k + 1) * D] = W.T.astype(bf)
    xqb = input_Q.astype(bf)
    xkb = input_K.astype(bf)
    xvb = input_V.astype(bf)
    return [
        {"xq": input_Q[b], "xqb": xqb[b], "xkb": xkb[b], "xvb": xvb[b],
         "wall": wall, "wallb": wallb}
        for b in range(B)
    ]


def kernel(input_Q, input_K, input_V, W_Q, W_K, W_V, W_fc, ln_gamma, ln_beta):
    input_Q = np.ascontiguousarray(np.asarray(input_Q, dtype=np.float32))
    input_K = np.ascontiguousarray(np.asarray(input_K, dtype=np.float32))
    input_V = np.ascontiguousarray(np.asarray(input_V, dtype=np.float32))
    W_Q = np.ascontiguousarray(np.asarray(W_Q, dtype=np.float32))
    W_K = np.ascontiguousarray(np.asarray(W_K, dtype=np.float32))
    W_V = np.ascontiguousarray(np.asarray(W_V, dtype=np.float32))
    W_fc = np.ascontiguousarray(np.asarray(W_fc, dtype=np.float32))
    ln_gamma = np.ascontiguousarray(np.asarray(ln_gamma, dtype=np.float32))
    ln_beta = np.ascontiguousarray(np.asarray(ln_beta, dtype=np.float32))

    apply_affine = not (np.all(ln_gamma == 1.0) and np.all(ln_beta == 0.0))

    key = ("nc", apply_affine)
    if key not in _CACHE:
        _CACHE[key] = _build(apply_affine)
    nc = _CACHE[key]

    in_maps = _pack_in_maps(input_Q, input_K, input_V, W_Q, W_K, W_V, W_fc,
                            ln_gamma, ln_beta)
    res = run_bass_kernel_spmd(nc, in_maps, core_ids=list(range(B)))
    return np.stack([res.results[b]["out"] for b in range(B)], axis=0)


# revision 27
# speedup vs baseline: 1.0097x; 1.0097x over previous
"""Trainium2 Bass kernel for a multi-head-attention block (B,C,N,D = 8,4,1024,96;
H=3 heads, dk=dv=32; softmax over the QUERY axis; residual + LayerNorm).

Sharding: pure data-parallel over batch B across 8 NeuronCores (one batch
element per core, C=4 channel-slices each, no collectives).

Schedule notes (v2):
  - exp work is split across engines: per chunk, heads 0/1 exp on ACT
    (spline exp + fused accumulator denominator + accumulator read), and
    head 2 of DVE-light slots runs a Schraudolph bit-hack exp on the DVE
    (one tensor_scalar: int16 = round(scores*A + B), bitcast bf16) with a
    deferred tensor_reduce denominator.  Per-tile constant factors cancel
    exactly in the q-axis softmax, so the ~4% pointwise exp error averages
    to ~1e-3 relative in the final output.
  - softmax normalization (vsc = V/denom, plus 1/denom write-back nobody
    reads) runs on the otherwise-idle GPSIMD via normalize_recip (attn
    ucode library), freeing the DVE.
  - PSUM pools are split so the scores ring (w_psum, 2 bufs x 2 banks)
    holds ONLY score tiles: the ring dependency is exp(i,h) -> scores MM,
    an ACT->PE edge with no DVE cast interposed.  Filler tiles
    (transpose/projection/v/fc) live in their own 2x1-bank pool (fp) and
    self-pace against their casts.  ctx accumulator: 2 banks.  8 total.
  - inputs arrive in DRAM twice: f32 xq (residual path) and host-rounded
    bf16 xq/xk/xv (projection path).  PE transposes are is_transpose
    matmuls with a bf16 identity writing bf16 PSUM (half the banks, one
    2x-mode cast per input instead of two 1x f32 casts).
  - lag-2 chunk pipeline: norm_recip(c,i-1) behind scores/exp(c,i), and
    ctx(c,i-2) one slot later so the PE never waits the gpsimd round trip;
    channel boundaries keep the same cadence.
  - LN tail: residual via scalar_tensor_tensor, bn_stats/bn_aggr for
    mean/var, rsqrt via bit-hack + 2 Newton iterations (all DVE);
    channel-3 tail is pipelined per q-half with ACT-based ln_out.
"""

from contextlib import ExitStack

import ml_dtypes
import numpy as np

import concourse.bass as bass
import concourse.tile as tile
from concourse import bacc, mybir
from concourse.bass_utils import run_bass_kernel_spmd

F32 = mybir.dt.float32
BF16 = mybir.dt.bfloat16
F32R = mybir.dt.float32r
I32 = mybir.dt.int32
I16 = mybir.dt.int16
A = mybir.AluOpType

B, C, N, D = 8, 4, 1024, 96
H, DK, DV = 3, 32, 32
P = 128               # partition size / token chunk
NCHUNK = N // P       # 8
QT = 512              # matmul free-dim limit into one PSUM bank (f32)
HN = N // 2
SCALE = 1.0 / np.sqrt(DK)
EPS = 1e-5

# Schraudolph exp on the DVE: bits(bf16 e) = round(x*SCALE*A + B) as int16.
SCH_A = float((2.0 ** 7) / np.log(2.0) * SCALE)
SCH_B = float(127 * 2 ** 7)


def _dve_tile(c, i):
    """Chunks whose head-2 exp runs on the DVE instead of ACT: the
    engine-balance knob, placed in DVE-light slots only.  The final chunk
    stays on ACT so the epilogue's normalization chain starts sooner."""
    if c == C - 1 and i == NCHUNK - 1:
        return False
    return (c >= 1 and i in (0, 1, 5, 7)) or (c == 0 and i in (1, 7))

_CACHE = {}


def _emit(nc, tc, ctx, apply_affine):
    xq_d = nc.dram_tensor("xq", [C, N, D], F32R, kind="ExternalInput").ap()
    xqb_d = nc.dram_tensor("xqb", [C, N, D], BF16, kind="ExternalInput").ap()
    xkb_d = nc.dram_tensor("xkb", [C, N, D], BF16, kind="ExternalInput").ap()
    xvb_d = nc.dram_tensor("xvb", [C, N, D], BF16, kind="ExternalInput").ap()
    # wall  = host-packed f32 [128, 96+2]: wfc.T (zero-padded) | gamma | beta
    # wallb = host-packed bf16 [128, 128 + 3*96]: identity | wq|wk|wv (each
    #         W.T, [96,96] natural, zero-padded to 128 rows)
    wall_d = nc.dram_tensor("wall", [P, D + 2], F32R,
                            kind="ExternalInput").ap()
    wallb_d = nc.dram_tensor("wallb", [P, P + 3 * D], BF16,
                             kind="ExternalInput").ap()
    out_d = nc.dram_tensor("out", [C, N, D], F32, kind="ExternalOutput").ap()

    const = ctx.enter_context(tc.tile_pool(name="const", bufs=1))
    pc = ctx.enter_context(tc.tile_pool(name="perc", bufs=2))
    w_psum = ctx.enter_context(tc.tile_pool(name="w_psum", bufs=3, space="PSUM"))
    ctx_psum = ctx.enter_context(tc.tile_pool(name="ctx_psum", bufs=1, space="PSUM"))

    # ---- w_psum ring discipline: pad allocations to multiples of 3 so the
    # 3 scores tiles of chunk i+1 land exactly on the banks freed by the
    # 3 exps of chunk i (same head -> earliest possible reuse).
    wct = {"n": 0, "pad": 0}

    def wtile(shape, dtype, name):
        wct["n"] += 1
        return w_psum.tile(shape, dtype, name=name, tag="w")

    def wpad():
        while wct["n"] % 3:
            wct["n"] += 1
            wct["pad"] += 1
            w_psum.tile([P, 8], F32, name=f"pad{wct['pad']}", tag="w")

    # ---------------- prologue: DMAs + PE warm-up spins ----------------
    dummy = const.tile([P, QT], BF16)
    nc.vector.memset(dummy, 0)

    # identity (gates the first transposes) before the weight columns
    wallb = const.tile([P, P + 3 * D], BF16)
    nc.sync.dma_start(out=wallb[:, 0:P], in_=wallb_d[:, 0:P])
    nc.sync.dma_start(out=wallb[:, P:], in_=wallb_d[:, P:])
    ident_b = wallb[:, 0:P]
    wb = {nm: wallb[0:D, P + k * D:P + (k + 1) * D]
          for k, nm in enumerate(("wq", "wk", "wv"))}

    # f32 side: wfc (first needed at fc_group(0) in slot (1,2)) + ln affine
    wall = const.tile([P, D + 2], F32R)
    nc.gpsimd.dma_start(out=wall, in_=wall_d)
    wfc = wall[0:D, 0:D]

    # channel-0 loads: bf16 projections path split for earliest q
    xq0 = pc.tile([P, NCHUNK, D], F32R, name="xq_nat0", tag="xq_nat", bufs=3)
    xqb0 = pc.tile([P, NCHUNK, D], BF16, name="xqb0", tag="xqb", bufs=1)
    xkb0 = pc.tile([P, NCHUNK, D], BF16, name="xkb0", tag="xkb", bufs=1)
    xvb0 = pc.tile([P, NCHUNK, D], BF16, name="xvb0", tag="xvb", bufs=1)
    xqb0_r = xqb_d[0].rearrange("(i p) d -> p i d", p=P)
    xkb0_r = xkb_d[0].rearrange("(i p) d -> p i d", p=P)
    # ACT's DMA queue is free before the first exp; one DMA per queue to
    # avoid per-queue DGE serialization
    nc.scalar.dma_start(out=xqb0[:, 0:4, :], in_=xqb0_r[:, 0:4, :])
    nc.gpsimd.dma_start(out=xqb0[:, 4:8, :], in_=xqb0_r[:, 4:8, :])
    nc.gpsimd.dma_start(out=xkb0[:, 0:4, :], in_=xkb0_r[:, 0:4, :])
    nc.gpsimd.dma_start(out=xkb0[:, 4:8, :], in_=xkb0_r[:, 4:8, :])
    nc.gpsimd.dma_start(out=xvb0, in_=xvb_d[0].rearrange("(i p) d -> p i d", p=P))
    nc.sync.dma_start(out=xq0, in_=xq_d[0].rearrange("(i p) d -> p i d", p=P))

    spin = wtile([P, N], F32, "spin")
    for _ in range(2):
        nc.tensor.matmul(spin[0:64, 0:256], lhsT=dummy[:, 0:64],
                         rhs=dummy[:, 0:256], start=True, stop=True,
                         skip_group_check=True)

    # warm up the gpsimd attn ucode library (normalize_recip): the inserted
    # MODIFY_POOL_CONFIG + ~6us IRAM load runs now, hidden under channel-0
    # compute, instead of stalling the first real norm_recip mid-pipeline
    warm = const.tile([P, 4], F32)
    nc.vector.memset(warm, 1.0)
    nc.gpsimd.normalize_recip(out_ap=warm[:, 2:3], in_ap=warm[:, 0:1],
                              denom_ap=warm[:, 1:2])

    gam_tile = bet_tile = None
    if apply_affine:
        gam_tile = const.tile([P, D], F32)
        bet_tile = const.tile([P, D], F32)
        for t, col in ((gam_tile, D), (bet_tile, D + 1)):
            col_ap = wall_d[0:D, col:col + 1]
            bcast = bass.AP(tensor=col_ap.tensor, offset=col_ap.offset,
                            ap=[[0, P], col_ap.ap[0]])
            nc.gpsimd.dma_start(out=t, in_=bcast)

    st = {0: dict(xq_nat=xq0, xqb=xqb0, xkb=xkb0, xvb=xvb0, xTs={})}
    ts = {}

    def alloc_chunk_state(c):
        s = st[c]
        s["ssum"] = pc.tile([P, H * NCHUNK], F32, name=f"ssum{c}", tag="ssum")
        s["e_all"] = pc.tile([P, H * NCHUNK, N], BF16, name=f"e{c}", tag="e")
        s["vsc_all"] = pc.tile([P, H * NCHUNK, DV], BF16, name=f"vsc{c}",
                               tag="vsc")

    def loads(c):
        """DMA loads for channel c (c >= 1): no triggers on the Scalar eng."""
        xq = pc.tile([P, NCHUNK, D], F32R, name=f"xq_nat{c}", tag="xq_nat",
                     bufs=3)
        xqb = pc.tile([P, NCHUNK, D], BF16, name=f"xqb{c}", tag="xqb", bufs=1)
        xkb = pc.tile([P, NCHUNK, D], BF16, name=f"xkb{c}", tag="xkb", bufs=1)
        xvb = pc.tile([P, NCHUNK, D], BF16, name=f"xvb{c}", tag="xvb", bufs=1)
        nc.sync.dma_start(out=xq, in_=xq_d[c].rearrange("(i p) d -> p i d", p=P))
        nc.sync.dma_start(out=xqb, in_=xqb_d[c].rearrange("(i p) d -> p i d", p=P))
        nc.gpsimd.dma_start(out=xkb, in_=xkb_d[c].rearrange("(i p) d -> p i d", p=P))
        nc.gpsimd.dma_start(out=xvb, in_=xvb_d[c].rearrange("(i p) d -> p i d", p=P))
        st[c] = dict(xq_nat=xq, xqb=xqb, xkb=xkb, xvb=xvb, xTs={})

    def tr(c, nm, g=None):
        """PE is_transpose (bf16 in/out PSUM) of input nm; g=None does all
        8 chunks with a single 2x-mode cast, g in {0,1} does half."""
        s = st[c]
        src = s[f"x{nm}b"]
        if nm not in s["xTs"]:
            s["xTs"][nm] = pc.tile([D, N], BF16, name=f"x{nm}T{c}",
                                   tag=f"x{nm}T", bufs=1)
        xT = s["xTs"][nm]
        if g is None:
            tp = wtile([D, N], BF16, f"tp{nm}{c}")
            for i in range(NCHUNK):
                nc.tensor.transpose(tp[:, i * P:(i + 1) * P],
                                    in_=src[:, i, :], identity=ident_b)
            nc.vector.tensor_copy(out=xT, in_=tp)
        else:
            tp = wtile([D, HN], BF16, f"tp{nm}{c}{g}")
            for j in range(4):
                i = 4 * g + j
                nc.tensor.transpose(tp[:, j * P:(j + 1) * P],
                                    in_=src[:, i, :], identity=ident_b)
            nc.vector.tensor_copy(out=xT[:, g * HN:(g + 1) * HN], in_=tp)

    def proj_qk(c, which, g, on_act=False):
        """Q or K projection into [e, tok] bf16 layout, qtile g.  on_act
        moves the PSUM->SBUF cast to the (idle) Scalar engine -- prologue
        only, to parallelize the cast chain to the first exp."""
        s = st[c]
        nm, w_t = (("qdT", wb["wq"]) if which == "q" else ("kdT", wb["wk"]))
        if nm not in s:
            s[nm] = pc.tile([D, N], BF16, name=f"{nm}{c}", tag=nm)
        dst, xT = s[nm], s["xTs"][which]
        pr_ps = wtile([D, QT], F32, f"pr{c}{which}{g}")
        nc.tensor.matmul(pr_ps, lhsT=w_t, rhs=xT[:, g * QT:(g + 1) * QT],
                         start=True, stop=True)
        if on_act:
            nc.scalar.copy(out=dst[:, g * QT:(g + 1) * QT], in_=pr_ps)
        else:
            nc.vector.tensor_copy(out=dst[:, g * QT:(g + 1) * QT], in_=pr_ps)

    def v_g(c, g):
        """V projection (natural f32 layout), chunks 4g..4g+3"""
        s = st[c]
        if "v_nat" not in s:
            s["v_nat"] = pc.tile([P, NCHUNK, D], F32, name=f"v_nat{c}",
                                 tag="v_nat")
        v_nat = s["v_nat"]
        v_ps = wtile([P, 4 * D], F32, f"vps{c}{g}")
        for j in range(4):
            i = 4 * g + j
            nc.tensor.matmul(v_ps[:, j * D:(j + 1) * D],
                             lhsT=s["xTs"]["v"][:, i * P:(i + 1) * P],
                             rhs=wb["wv"], start=True, stop=True)
        nc.vector.tensor_copy(
            out=v_nat[:, 4 * g:4 * (g + 1), :].rearrange("p i d -> p (i d)"),
            in_=v_ps)

    pending_reduce = []

    def exp_tile(c, i, h):
        """exp for one (chunk, head) score tile.  ACT: spline exp + fused
        accumulator.  DVE: Schraudolph tensor_scalar; its tensor_reduce
        denominator is deferred to the slot end (a full slot of slack
        before the gpsimd norm_recip needs it)."""
        s = st[c]
        j = i * H + h
        if h == 2 and _dve_tile(c, i):
            nc.vector.tensor_scalar(
                out=s["e_all"][:, j, :].bitcast(I16), in0=s["s_regs"][h],
                scalar1=SCH_A, scalar2=SCH_B, op0=A.mult, op1=A.add)
            pending_reduce.append((c, j))
        else:
            nc.scalar.activation(
                out=s["e_all"][:, j, :], in_=s["s_regs"][h],
                func=mybir.ActivationFunctionType.Exp,
                scale=SCALE, accum_out=s["ssum"][:, j:j + 1])

    def flush_reduce():
        while pending_reduce:
            c_, j_ = pending_reduce.pop(0)
            s_ = st[c_]
            nc.vector.tensor_reduce(
                out=s_["ssum"][:, j_:j_ + 1], in_=s_["e_all"][:, j_, :],
                axis=mybir.AxisListType.X, op=A.add)

    def scores_exp(c, i):
        """S_T + exp for chunk i.  The three heads' matmuls are adjacent at
        row-groups 0/32/64 so they run concurrently in the PE array."""
        s = st[c]
        s_regs = [wtile([P, N], F32, f"s{c}_{i}_{h}") for h in range(H)]
        s["s_regs"] = s_regs
        for g in range(2):
            for h in range(H):
                hs = slice(DK * h, DK * (h + 1))
                nc.tensor.matmul(
                    s_regs[h][:, g * QT:(g + 1) * QT],
                    lhsT=s["kdT"][hs, i * P:(i + 1) * P],
                    rhs=s["qdT"][hs, g * QT:(g + 1) * QT],
                    start=True, stop=True)
        for h in range(H):
            exp_tile(c, i, h)

    def vsc_one(c, i, h):
        s = st[c]
        hs = slice(DV * h, DV * (h + 1))
        j = i * H + h
        nc.gpsimd.normalize_recip(
            out_ap=s["vsc_all"][:, j, :], in_ap=s["v_nat"][:, i, hs],
            denom_ap=s["ssum"][:, j:j + 1])

    def vsc_mm(c, i, heads=range(H)):
        """fold 1/denom into V rows for chunk i: gpsimd normalize_recip
        (vsc = v/denom; ssum overwritten with 1/denom, unread)."""
        for h in heads:
            vsc_one(c, i, h)

    def ctx_mm(c, i):
        """context accumulation for chunk i: bf16, three heads at
        col-groups 0/32/64, emitted adjacently -> concurrent."""
        s = st[c]
        for g in range(2):
            for h in range(H):
                hs = slice(DV * h, DV * (h + 1))
                j = i * H + h
                nc.tensor.matmul(
                    s["ctx_ps"][hs, g * QT:(g + 1) * QT],
                    lhsT=s["vsc_all"][:, j, :],
                    rhs=s["e_all"][:, j, g * QT:(g + 1) * QT],
                    start=(i == 0), stop=(i == NCHUNK - 1),
                    skip_group_check=True)

    def tail_a(c, half=None):
        """ctx copy-out (frees the ctx PSUM banks); half in {0,1,None}"""
        s = st[c]
        if "ctxT" not in s:
            s["ctxT"] = pc.tile([D, N], F32R, name=f"ctxT{c}", tag="ctxT")
        halves = (0, 1) if half is None else (half,)
        for g in halves:
            nc.vector.tensor_copy(out=s["ctxT"][:, g * QT:(g + 1) * QT],
                                  in_=s["ctx_ps"][:, g * QT:(g + 1) * QT])
        if half in (1, None):
            del s["ctx_ps"]

    def fc_group(c, g):
        """fc matmuls + residual add + one bn_stats pass for chunks 4g..4g+3"""
        s = st[c]
        if c not in ts:
            ts[c] = dict(
                t_all=pc.tile([P, NCHUNK, D], F32, name=f"tall{c}", tag="tall"),
                bst=pc.tile([P, NCHUNK, 6], F32, name=f"bst{c}", tag="bst"),
                mv=pc.tile([P, NCHUNK, 2], F32, name=f"mv{c}", tag="mv"),
            )
        t = ts[c]
        fc_ps = wtile([P, 4 * D], F32, f"fc{c}{g}")
        for j in range(4):
            i = 4 * g + j
            nc.tensor.matmul(fc_ps[:, j * D:(j + 1) * D],
                             lhsT=s["ctxT"][:, i * P:(i + 1) * P],
                             rhs=wfc, start=True, stop=True)
        for j in range(4):
            i = 4 * g + j
            nc.vector.scalar_tensor_tensor(
                out=t["t_all"][:, i, :], in0=fc_ps[:, j * D:(j + 1) * D],
                scalar=1.0, in1=s["xq_nat"][:, i, :].bitcast(F32),
                op0=A.mult, op1=A.add)
            nc.vector.bn_stats(out=t["bst"][:, i, :], in_=t["t_all"][:, i, :])

    def ln_stats(c, lo=0, hi=NCHUNK):
        """per-chunk bn_aggr -> (mean, var); rstd via bit-hack + 2 Newton
        iterations (DVE)"""
        t = ts[c]
        if "var" not in t:
            t["var"] = pc.tile([P, NCHUNK], F32, name=f"var{c}", tag="var")
            t["y"] = pc.tile([P, NCHUNK], F32, name=f"y{c}", tag="y")
            t["t1"] = pc.tile([P, NCHUNK], F32, name=f"t1n{c}", tag="t1n")
        sl = slice(lo, hi)
        var, y, t1 = t["var"][:, sl], t["y"][:, sl], t["t1"][:, sl]
        for i in range(lo, hi):
            nc.vector.bn_aggr(out=t["mv"][:, i, :], in_=t["bst"][:, i, :])
        nc.vector.tensor_scalar_add(
            out=var, in0=t["mv"][:, sl, 1:2].rearrange("p i o -> p (i o)"),
            scalar1=EPS)
        nc.vector.tensor_scalar(
            out=y.bitcast(I32), in0=var.bitcast(I32), scalar1=1,
            scalar2=None, op0=A.logical_shift_right)
        nc.vector.tensor_scalar(
            out=y.bitcast(I32), in0=y.bitcast(I32), scalar1=-1,
            scalar2=None, op0=A.bitwise_xor)
        nc.vector.tensor_scalar(
            out=y.bitcast(I32), in0=y.bitcast(I32), scalar1=0x5F3759E0,
            scalar2=None, op0=A.add)
        for _ in range(2):
            nc.vector.tensor_mul(out=t1, in0=y, in1=y)
            nc.vector.tensor_mul(out=t1, in0=t1, in1=var)
            nc.vector.tensor_scalar(out=t1, in0=t1, scalar1=-0.5, scalar2=1.5,
                                    op0=A.mult, op1=A.add)
            nc.vector.tensor_mul(out=y, in0=y, in1=t1)

    def ln_out(c, lo, hi, on_act=False):
        """(t - mean) * rstd, chunks lo..hi-1.  on_act runs it on the
        Scalar engine as t*rstd + (-mean*rstd) - only useful in the
        epilogue when the exps are finished and ACT is idle."""
        t = ts[c]
        if "out_sb" not in t:
            t["out_sb"] = pc.tile([P, NCHUNK, D], F32, name=f"osb{c}",
                                  tag="osb")
        if on_act:
            if "nmb" not in t:
                t["nmb"] = pc.tile([P, NCHUNK], F32, name=f"nmb{c}",
                                   tag="nmb")
            nc.vector.scalar_tensor_tensor(
                out=t["nmb"][:, lo:hi],
                in0=t["mv"][:, lo:hi, 0:1].rearrange("p i o -> p (i o)"),
                scalar=-1.0, in1=t["y"][:, lo:hi], op0=A.mult, op1=A.mult)
            for i in range(lo, hi):
                nc.scalar.activation(
                    out=t["out_sb"][:, i, :], in_=t["t_all"][:, i, :],
                    func=mybir.ActivationFunctionType.Identity,
                    scale=t["y"][:, i:i + 1], bias=t["nmb"][:, i:i + 1])
            return
        for i in range(lo, hi):
            nc.vector.tensor_scalar(
                out=t["out_sb"][:, i, :], in0=t["t_all"][:, i, :],
                scalar1=t["mv"][:, i, 0:1], scalar2=t["y"][:, i:i + 1],
                op0=A.subtract, op1=A.mult)
        if apply_affine:
            for i in range(lo, hi):
                nc.vector.tensor_mul(out=t["out_sb"][:, i, :],
                                     in0=t["out_sb"][:, i, :], in1=gam_tile)
                nc.vector.tensor_add(out=t["out_sb"][:, i, :],
                                     in0=t["out_sb"][:, i, :], in1=bet_tile)

    def store(c, half=None):
        t = ts[c]
        out_r = out_d[c].rearrange("(i p) d -> p i d", p=P)
        if half is None:
            nc.sync.dma_start(out=out_r, in_=t["out_sb"])
        elif half == 0:
            nc.sync.dma_start(out=out_r[:, 0:4, :], in_=t["out_sb"][:, 0:4, :])
        else:
            nc.sync.dma_start(out=out_r[:, 4:8, :], in_=t["out_sb"][:, 4:8, :])

    # ------------- channel-0 minimal critical path to first exp -------------
    alloc_chunk_state(0)
    tr(0, "q", 0)
    tr(0, "q", 1)
    proj_qk(0, "q", 0, on_act=True)
    proj_qk(0, "q", 1)
    tr(0, "k", 0)
    proj_qk(0, "k", 0, on_act=True)
    wpad()
    scores_exp(0, 0)

    # per-slot fillers; emitted AFTER the slot's scores+ctx matmuls.
    def fillers(c, i):
        if c == 0:
            if i == 0:
                tr(0, "k", 1)
                proj_qk(0, "k", 1)
                tr(0, "v", 0)
                v_g(0, 0)
            elif i == 1:
                tr(0, "v", 1)
                v_g(0, 1)
                loads(1)
            elif i == 2:
                tr(1, "q")
            elif i == 3:
                tr(1, "k")
            elif i == 4:
                tr(1, "v")
            elif i == 5:
                proj_qk(1, "q", 0)
            elif i == 6:
                proj_qk(1, "q", 1)
                proj_qk(1, "k", 0)
                proj_qk(1, "k", 1)
            # i == 7 left empty so scores(1, 0) is never gated on a filler
            # cast stuck behind slot-7 DVE work
        else:
            # lag-2 ctx pipeline: ctx(c-1, 7) lands in slot (c, 1), so the
            # previous channel's tail shifts one slot later
            if i == 0:
                v_g(c, 0)
                v_g(c, 1)
                if c + 1 < C:
                    loads(c + 1)
            elif i == 1:
                tail_a(c - 1)
            elif i == 2:
                if c + 1 < C:
                    tr(c + 1, "q")
                fc_group(c - 1, 0)
            elif i == 3:
                if c + 1 < C:
                    tr(c + 1, "k")
                fc_group(c - 1, 1)
            elif i == 4:
                if c + 1 < C:
                    tr(c + 1, "v")
                ln_stats(c - 1)
            elif i == 5:
                if c + 1 < C:
                    proj_qk(c + 1, "q", 0)
                ln_out(c - 1, 0, 4)
                store(c - 1, half=0)
            elif i == 6:
                if c + 1 < C:
                    proj_qk(c + 1, "q", 1)
                    proj_qk(c + 1, "k", 0)
                    proj_qk(c + 1, "k", 1)
                ln_out(c - 1, 4, NCHUNK)
                store(c - 1, half=1)
            # i == 7 left empty (see channel-0 note)

    # ---------------- software-pipelined channel loop ----------------
    # lag-2 chunk pipeline (see docstring)
    for c in range(C):
        if c > 0:
            alloc_chunk_state(c)
        st[c]["ctx_ps"] = ctx_psum.tile([D, N], F32, name=f"ctx{c}", tag="ctx")
        for i in range(NCHUNK):
            if not (c == 0 and i == 0):
                scores_exp(c, i)
            pc_, pi = (c, i - 1) if i >= 1 else (c - 1, NCHUNK - 1)
            if pc_ >= 0:
                vsc_mm(pc_, pi, heads=(0, 1))
            if i >= 2:
                ctx_mm(c, i - 2)
            elif c >= 1 and i == 0:
                ctx_mm(c - 1, NCHUNK - 2)
            elif c >= 1 and i == 1:
                ctx_mm(c - 1, NCHUNK - 1)
            fillers(c, i)
            # deferred Schraudolph denominator at slot end (no downstream
            # pressure: its norm_recip feeds ctx a full slot later), then
            # the head-2 normalization of the previous chunk
            flush_reduce()
            if pc_ >= 0:
                vsc_mm(pc_, pi, heads=(2,))
            wpad()

    # ---------------- pipelined epilogue for channel 3 ----------------
    c = C - 1
    s, li = st[c], NCHUNK - 1
    flush_reduce()
    ctx_mm(c, NCHUNK - 2)
    for h in range(H):
        j = li * H + h
        hs = slice(DV * h, DV * (h + 1))
        nc.gpsimd.normalize_recip(
            out_ap=s["vsc_all"][:, j, :], in_ap=s["v_nat"][:, li, hs],
            denom_ap=s["ssum"][:, j:j + 1])
        for g in range(2):
            nc.tensor.matmul(
                s["ctx_ps"][hs, g * QT:(g + 1) * QT],
                lhsT=s["vsc_all"][:, j, :],
                rhs=s["e_all"][:, j, g * QT:(g + 1) * QT],
                start=False, stop=True, skip_group_check=True)
    tail_a(c, half=0)
    fc_group(c, 0)
    tail_a(c, half=1)
    ln_stats(c, 0, 4)
    fc_group(c, 1)
    out_r = out_d[c].rearrange("(i p) d -> p i d", p=P)
    t3 = None
    for lo in range(0, NCHUNK, 2):
        if lo == 4:
            ln_stats(c, 4, NCHUNK)
        ln_out(c, lo, lo + 2, on_act=True)
        t3 = ts[c]
        nc.sync.dma_start(out=out_r[:, lo:lo + 2, :],
                          in_=t3["out_sb"][:, lo:lo + 2, :])


def _build(apply_affine):
    nc = bacc.Bacc("TRN2", target_bir_lowering=False, debug=False, num_devices=B)
    with tile.TileContext(nc) as tc, ExitStack() as ctx:
        _emit(nc, tc, ctx, apply_affine)
    nc.compile()
    return nc


def _pack_in_maps(input_Q, input_K, input_V, W_Q, W_K, W_V, W_fc,
                  ln_gamma, ln_beta):
    bf = ml_dtypes.bfloat16
    wall = np.zeros((P, D + 2), dtype=np.float32)
    wall[0:D, 0:D] = W_fc.T
    wall[0:D, D] = ln_gamma
    wall[0:D, D + 1] = ln_beta
    wallb = np.zeros((P, P + 3 * D), dtype=bf)
    wallb[:, :P] = np.eye(P, dtype=np.float32).astype(bf)
    for k, W in enumerate((W_Q, W_K, W_V)):
        wallb[0:D, P + k * D:P + (k + 1) * D] = W.T.astype(bf)
    xqb = input_Q.astype(bf)
    xkb = input_K.astype(bf)
    xvb = input_V.astype(bf)
    return [
        {"xq": input_Q[b], "xqb": xqb[b], "xkb": xkb[b], "xvb": xvb[b],
         "wall": wall, "wallb": wallb}
        for b in range(B)
    ]


def kernel(input_Q, input_K, input_V, W_Q, W_K, W_V, W_fc, ln_gamma, ln_beta):
    input_Q = np.ascontiguousarray(np.asarray(input_Q, dtype=np.float32))
    input_K = np.ascontiguousarray(np.asarray(input_K, dtype=np.float32))
    input_V = np.ascontiguousarray(np.asarray(input_V, dtype=np.float32))
    W_Q = np.ascontiguousarray(np.asarray(W_Q, dtype=np.float32))
    W_K = np.ascontiguousarray(np.asarray(W_K, dtype=np.float32))
    W_V = np.ascontiguousarray(np.asarray(W_V, dtype=np.float32))
    W_fc = np.ascontiguousarray(np.asarray(W_fc, dtype=np.float32))
    ln_gamma = np.ascontiguousarray(np.asarray(ln_gamma, dtype=np.float32))
    ln_beta = np.ascontiguousarray(np.asarray(ln_beta, dtype=np.float32))

    apply_affine = not (np.all(ln_gamma == 1.0) and np.all(ln_beta == 0.0))

    key = ("nc", apply_affine)
    if key not in _CACHE:
        _CACHE[key] = _build(apply_affine)
    nc = _CACHE[key]

    in_maps = _pack_in_maps(input_Q, input_K, input_V, W_Q, W_K, W_V, W_fc,
                            ln_gamma, ln_beta)
    res = run_bass_kernel_spmd(nc, in_maps, core_ids=list(range(B)))
    return np.stack([res.results[b]["out"] for b in range(B)], axis=0)


# revision 28
# speedup vs baseline: 1.0401x; 1.0301x over previous
"""Trainium2 Bass kernel for a multi-head-attention block (B,C,N,D = 8,4,1024,96;
H=3 heads, dk=dv=32; softmax over the QUERY axis; residual + LayerNorm).

Sharding: pure data-parallel over batch B across 8 NeuronCores (one batch
element per core, C=4 channel-slices each, no collectives).

Schedule notes (v2):
  - exp work is split across engines: per chunk, heads 0/1 exp on ACT
    (spline exp + fused accumulator denominator + accumulator read), and
    head 2 of DVE-light slots runs a Schraudolph bit-hack exp on the DVE
    (one tensor_scalar: int16 = round(scores*A + B), bitcast bf16) with a
    deferred tensor_reduce denominator.  Per-tile constant factors cancel
    exactly in the q-axis softmax, so the ~4% pointwise exp error averages
    to ~1e-3 relative in the final output.
  - softmax normalization (vsc = V/denom, plus 1/denom write-back nobody
    reads) runs on the otherwise-idle GPSIMD via normalize_recip (attn
    ucode library), freeing the DVE.
  - PSUM pools are split so the scores ring (w_psum, 2 bufs x 2 banks)
    holds ONLY score tiles: the ring dependency is exp(i,h) -> scores MM,
    an ACT->PE edge with no DVE cast interposed.  Filler tiles
    (transpose/projection/v/fc) live in their own 2x1-bank pool (fp) and
    self-pace against their casts.  ctx accumulator: 2 banks.  8 total.
  - inputs arrive in DRAM twice: f32 xq (residual path) and host-rounded
    bf16 xq/xk/xv (projection path).  PE transposes are is_transpose
    matmuls with a bf16 identity writing bf16 PSUM (half the banks, one
    2x-mode cast per input instead of two 1x f32 casts).
  - lag-2 chunk pipeline: norm_recip(c,i-1) behind scores/exp(c,i), and
    ctx(c,i-2) one slot later so the PE never waits the gpsimd round trip;
    channel boundaries keep the same cadence.
  - LN tail: residual via scalar_tensor_tensor, bn_stats/bn_aggr for
    mean/var, rsqrt via bit-hack + 2 Newton iterations (all DVE);
    channel-3 tail is pipelined per q-half with ACT-based ln_out.
"""

from contextlib import ExitStack

import ml_dtypes
import numpy as np

import concourse.bass as bass
import concourse.tile as tile
from concourse import bacc, mybir
from concourse.bass_utils import run_bass_kernel_spmd

F32 = mybir.dt.float32
BF16 = mybir.dt.bfloat16
F32R = mybir.dt.float32r
I32 = mybir.dt.int32
I16 = mybir.dt.int16
A = mybir.AluOpType

B, C, N, D = 8, 4, 1024, 96
H, DK, DV = 3, 32, 32
P = 128               # partition size / token chunk
NCHUNK = N // P       # 8
QT = 512              # matmul free-dim limit into one PSUM bank (f32)
HN = N // 2
SCALE = 1.0 / np.sqrt(DK)
EPS = 1e-5

# Schraudolph exp on the DVE: bits(bf16 e) = round(x*SCALE*A + B) as int16.
SCH_A = float((2.0 ** 7) / np.log(2.0) * SCALE)
SCH_B = float(127 * 2 ** 7)


def _dve_tile(c, i):
    """Chunks whose head-2 exp runs on the DVE instead of ACT: the
    engine-balance knob, placed in DVE-light slots only.  The final chunk
    stays on ACT so the epilogue's normalization chain starts sooner."""
    if c == C - 1 and i == NCHUNK - 1:
        return False
    return (c >= 1 and i in (0, 1, 5, 7)) or (c == 0 and i in (1, 7))

_CACHE = {}


def _emit(nc, tc, ctx, apply_affine):
    xq_d = nc.dram_tensor("xq", [C, N, D], F32R, kind="ExternalInput").ap()
    xqb_d = nc.dram_tensor("xqb", [C, N, D], BF16, kind="ExternalInput").ap()
    xkb_d = nc.dram_tensor("xkb", [C, N, D], BF16, kind="ExternalInput").ap()
    xvb_d = nc.dram_tensor("xvb", [C, N, D], BF16, kind="ExternalInput").ap()
    # wall  = host-packed f32 [128, 96+2]: wfc.T (zero-padded) | gamma | beta
    # wallb = host-packed bf16 [128, 128 + 3*96]: identity | wq|wk|wv (each
    #         W.T, [96,96] natural, zero-padded to 128 rows)
    wall_d = nc.dram_tensor("wall", [P, D + 2], F32R,
                            kind="ExternalInput").ap()
    wallb_d = nc.dram_tensor("wallb", [P, P + 3 * D], BF16,
                             kind="ExternalInput").ap()
    out_d = nc.dram_tensor("out", [C, N, D], F32, kind="ExternalOutput").ap()

    const = ctx.enter_context(tc.tile_pool(name="const", bufs=1))
    pc = ctx.enter_context(tc.tile_pool(name="perc", bufs=2))
    w_psum = ctx.enter_context(tc.tile_pool(name="w_psum", bufs=3, space="PSUM"))
    ctx_psum = ctx.enter_context(tc.tile_pool(name="ctx_psum", bufs=1, space="PSUM"))

    # ---- w_psum ring discipline: pad allocations to multiples of 3 so the
    # 3 scores tiles of chunk i+1 land exactly on the banks freed by the
    # 3 exps of chunk i (same head -> earliest possible reuse).
    wct = {"n": 0, "pad": 0}

    def wtile(shape, dtype, name):
        wct["n"] += 1
        return w_psum.tile(shape, dtype, name=name, tag="w")

    def wpad():
        while wct["n"] % 3:
            wct["n"] += 1
            wct["pad"] += 1
            w_psum.tile([P, 8], F32, name=f"pad{wct['pad']}", tag="w")

    # ---------------- prologue: DMAs + PE warm-up spins ----------------
    dummy = const.tile([P, QT], BF16)
    nc.vector.memset(dummy, 0)

    # identity (gates the first transposes) before the weight columns
    wallb = const.tile([P, P + 3 * D], BF16)
    nc.sync.dma_start(out=wallb[:, 0:P], in_=wallb_d[:, 0:P])
    nc.sync.dma_start(out=wallb[:, P:], in_=wallb_d[:, P:])
    ident_b = wallb[:, 0:P]
    wb = {nm: wallb[0:D, P + k * D:P + (k + 1) * D]
          for k, nm in enumerate(("wq", "wk", "wv"))}

    # f32 side: wfc (first needed at fc_group(0) in slot (1,2)) + ln affine
    wall = const.tile([P, D + 2], F32R)
    nc.gpsimd.dma_start(out=wall, in_=wall_d)
    wfc = wall[0:D, 0:D]

    # channel-0 loads: bf16 projections path split for earliest q
    xq0 = pc.tile([P, NCHUNK, D], F32R, name="xq_nat0", tag="xq_nat", bufs=3)
    xqb0 = pc.tile([P, NCHUNK, D], BF16, name="xqb0", tag="xqb", bufs=1)
    xkb0 = pc.tile([P, NCHUNK, D], BF16, name="xkb0", tag="xkb", bufs=1)
    xvb0 = pc.tile([P, NCHUNK, D], BF16, name="xvb0", tag="xvb", bufs=1)
    xqb0_r = xqb_d[0].rearrange("(i p) d -> p i d", p=P)
    xkb0_r = xkb_d[0].rearrange("(i p) d -> p i d", p=P)
    # ACT's DMA queue is free before the first exp; one DMA per queue to
    # avoid per-queue DGE serialization
    nc.scalar.dma_start(out=xqb0[:, 0:4, :], in_=xqb0_r[:, 0:4, :])
    nc.gpsimd.dma_start(out=xqb0[:, 4:8, :], in_=xqb0_r[:, 4:8, :])
    nc.gpsimd.dma_start(out=xkb0[:, 0:4, :], in_=xkb0_r[:, 0:4, :])
    nc.gpsimd.dma_start(out=xkb0[:, 4:8, :], in_=xkb0_r[:, 4:8, :])
    nc.gpsimd.dma_start(out=xvb0, in_=xvb_d[0].rearrange("(i p) d -> p i d", p=P))
    nc.sync.dma_start(out=xq0, in_=xq_d[0].rearrange("(i p) d -> p i d", p=P))

    spin = wtile([P, N], F32, "spin")
    for _ in range(2):
        nc.tensor.matmul(spin[0:64, 0:256], lhsT=dummy[:, 0:64],
                         rhs=dummy[:, 0:256], start=True, stop=True,
                         skip_group_check=True)

    # warm up the gpsimd attn ucode library (normalize_recip): the inserted
    # MODIFY_POOL_CONFIG + ~6us IRAM load runs now, hidden under channel-0
    # compute, instead of stalling the first real norm_recip mid-pipeline
    warm = const.tile([P, 4], F32)
    nc.vector.memset(warm, 1.0)
    nc.gpsimd.normalize_recip(out_ap=warm[:, 2:3], in_ap=warm[:, 0:1],
                              denom_ap=warm[:, 1:2])

    gam_tile = bet_tile = None
    if apply_affine:
        gam_tile = const.tile([P, D], F32)
        bet_tile = const.tile([P, D], F32)
        for t, col in ((gam_tile, D), (bet_tile, D + 1)):
            col_ap = wall_d[0:D, col:col + 1]
            bcast = bass.AP(tensor=col_ap.tensor, offset=col_ap.offset,
                            ap=[[0, P], col_ap.ap[0]])
            nc.gpsimd.dma_start(out=t, in_=bcast)

    st = {0: dict(xq_nat=xq0, xqb=xqb0, xkb=xkb0, xvb=xvb0, xTs={})}
    ts = {}

    def alloc_chunk_state(c):
        s = st[c]
        s["ssum"] = pc.tile([P, H * NCHUNK], F32, name=f"ssum{c}", tag="ssum")
        s["e_all"] = pc.tile([P, H * NCHUNK, N], BF16, name=f"e{c}", tag="e")
        s["vsc_all"] = pc.tile([P, H * NCHUNK, DV], BF16, name=f"vsc{c}",
                               tag="vsc")

    def loads(c):
        """DMA loads for channel c (c >= 1): no triggers on the Scalar eng."""
        xq = pc.tile([P, NCHUNK, D], F32R, name=f"xq_nat{c}", tag="xq_nat",
                     bufs=3)
        xqb = pc.tile([P, NCHUNK, D], BF16, name=f"xqb{c}", tag="xqb", bufs=1)
        xkb = pc.tile([P, NCHUNK, D], BF16, name=f"xkb{c}", tag="xkb", bufs=1)
        xvb = pc.tile([P, NCHUNK, D], BF16, name=f"xvb{c}", tag="xvb", bufs=1)
        nc.sync.dma_start(out=xq, in_=xq_d[c].rearrange("(i p) d -> p i d", p=P))
        nc.sync.dma_start(out=xqb, in_=xqb_d[c].rearrange("(i p) d -> p i d", p=P))
        nc.gpsimd.dma_start(out=xkb, in_=xkb_d[c].rearrange("(i p) d -> p i d", p=P))
        nc.gpsimd.dma_start(out=xvb, in_=xvb_d[c].rearrange("(i p) d -> p i d", p=P))
        st[c] = dict(xq_nat=xq, xqb=xqb, xkb=xkb, xvb=xvb, xTs={})

    def tr(c, nm, g=None):
        """PE is_transpose (bf16 in/out PSUM) of input nm; g=None does all
        8 chunks with a single 2x-mode cast, g in {0,1} does half."""
        s = st[c]
        src = s[f"x{nm}b"]
        if nm not in s["xTs"]:
            s["xTs"][nm] = pc.tile([D, N], BF16, name=f"x{nm}T{c}",
                                   tag=f"x{nm}T", bufs=1)
        xT = s["xTs"][nm]
        if g is None:
            tp = wtile([D, N], BF16, f"tp{nm}{c}")
            for i in range(NCHUNK):
                nc.tensor.transpose(tp[:, i * P:(i + 1) * P],
                                    in_=src[:, i, :], identity=ident_b)
            nc.vector.tensor_copy(out=xT, in_=tp)
        else:
            tp = wtile([D, HN], BF16, f"tp{nm}{c}{g}")
            for j in range(4):
                i = 4 * g + j
                nc.tensor.transpose(tp[:, j * P:(j + 1) * P],
                                    in_=src[:, i, :], identity=ident_b)
            nc.vector.tensor_copy(out=xT[:, g * HN:(g + 1) * HN], in_=tp)

    def proj_qk(c, which, g, on_act=False):
        """Q or K projection into [e, tok] bf16 layout, qtile g.  on_act
        moves the PSUM->SBUF cast to the (idle) Scalar engine -- prologue
        only, to parallelize the cast chain to the first exp."""
        s = st[c]
        nm, w_t = (("qdT", wb["wq"]) if which == "q" else ("kdT", wb["wk"]))
        if nm not in s:
            s[nm] = pc.tile([D, N], BF16, name=f"{nm}{c}", tag=nm)
        dst, xT = s[nm], s["xTs"][which]
        pr_ps = wtile([D, QT], F32, f"pr{c}{which}{g}")
        nc.tensor.matmul(pr_ps, lhsT=w_t, rhs=xT[:, g * QT:(g + 1) * QT],
                         start=True, stop=True)
        if on_act:
            nc.scalar.copy(out=dst[:, g * QT:(g + 1) * QT], in_=pr_ps)
        else:
            nc.vector.tensor_copy(out=dst[:, g * QT:(g + 1) * QT], in_=pr_ps)

    def v_g(c, g):
        """V projection (natural f32 layout), chunks 4g..4g+3"""
        s = st[c]
        if "v_nat" not in s:
            s["v_nat"] = pc.tile([P, NCHUNK, D], F32, name=f"v_nat{c}",
                                 tag="v_nat")
        v_nat = s["v_nat"]
        v_ps = wtile([P, 4 * D], F32, f"vps{c}{g}")
        for j in range(4):
            i = 4 * g + j
            nc.tensor.matmul(v_ps[:, j * D:(j + 1) * D],
                             lhsT=s["xTs"]["v"][:, i * P:(i + 1) * P],
                             rhs=wb["wv"], start=True, stop=True)
        nc.vector.tensor_copy(
            out=v_nat[:, 4 * g:4 * (g + 1), :].rearrange("p i d -> p (i d)"),
            in_=v_ps)

    pending_reduce = []

    def exp_tile(c, i, h):
        """exp for one (chunk, head) score tile.  ACT: spline exp + fused
        accumulator.  DVE: Schraudolph tensor_scalar; its tensor_reduce
        denominator is deferred to the slot end (a full slot of slack
        before the gpsimd norm_recip needs it)."""
        s = st[c]
        j = i * H + h
        if h == 2 and _dve_tile(c, i):
            nc.vector.tensor_scalar(
                out=s["e_all"][:, j, :].bitcast(I16), in0=s["s_regs"][h],
                scalar1=SCH_A, scalar2=SCH_B, op0=A.mult, op1=A.add)
            pending_reduce.append((c, j))
        else:
            nc.scalar.activation(
                out=s["e_all"][:, j, :], in_=s["s_regs"][h],
                func=mybir.ActivationFunctionType.Exp,
                scale=SCALE, accum_out=s["ssum"][:, j:j + 1])

    def flush_reduce():
        while pending_reduce:
            c_, j_ = pending_reduce.pop(0)
            s_ = st[c_]
            nc.vector.tensor_reduce(
                out=s_["ssum"][:, j_:j_ + 1], in_=s_["e_all"][:, j_, :],
                axis=mybir.AxisListType.X, op=A.add)

    def scores_exp(c, i):
        """S_T + exp for chunk i.  The three heads' matmuls are adjacent at
        row-groups 0/32/64 so they run concurrently in the PE array."""
        s = st[c]
        s_regs = [wtile([P, N], F32, f"s{c}_{i}_{h}") for h in range(H)]
        s["s_regs"] = s_regs
        for g in range(2):
            for h in range(H):
                hs = slice(DK * h, DK * (h + 1))
                nc.tensor.matmul(
                    s_regs[h][:, g * QT:(g + 1) * QT],
                    lhsT=s["kdT"][hs, i * P:(i + 1) * P],
                    rhs=s["qdT"][hs, g * QT:(g + 1) * QT],
                    start=True, stop=True)
        for h in range(H):
            exp_tile(c, i, h)

    def vsc_one(c, i, h):
        s = st[c]
        hs = slice(DV * h, DV * (h + 1))
        j = i * H + h
        nc.gpsimd.normalize_recip(
            out_ap=s["vsc_all"][:, j, :], in_ap=s["v_nat"][:, i, hs],
            denom_ap=s["ssum"][:, j:j + 1])

    def vsc_mm(c, i, heads=range(H)):
        """fold 1/denom into V rows for chunk i: gpsimd normalize_recip
        (vsc = v/denom; ssum overwritten with 1/denom, unread)."""
        for h in heads:
            vsc_one(c, i, h)

    def ctx_mm(c, i):
        """context accumulation for chunk i: bf16, three heads at
        col-groups 0/32/64, emitted adjacently -> concurrent."""
        s = st[c]
        # head-major with head 2 last: its vsc lands latest (end of the
        # previous slot), so its matmuls must not head-of-line-block the
        # other heads' ctx or anything behind them in the PE queue
        for h in (0, 1, 2):
            for g in range(2):
                hs = slice(DV * h, DV * (h + 1))
                j = i * H + h
                nc.tensor.matmul(
                    s["ctx_ps"][hs, g * QT:(g + 1) * QT],
                    lhsT=s["vsc_all"][:, j, :],
                    rhs=s["e_all"][:, j, g * QT:(g + 1) * QT],
                    start=(i == 0), stop=(i == NCHUNK - 1),
                    skip_group_check=True)

    def tail_a(c, half=None):
        """ctx copy-out (frees the ctx PSUM banks); half in {0,1,None}"""
        s = st[c]
        if "ctxT" not in s:
            s["ctxT"] = pc.tile([D, N], F32R, name=f"ctxT{c}", tag="ctxT")
        halves = (0, 1) if half is None else (half,)
        for g in halves:
            nc.vector.tensor_copy(out=s["ctxT"][:, g * QT:(g + 1) * QT],
                                  in_=s["ctx_ps"][:, g * QT:(g + 1) * QT])
        if half in (1, None):
            del s["ctx_ps"]

    def fc_group(c, g):
        """fc matmuls + residual add + one bn_stats pass for chunks 4g..4g+3"""
        s = st[c]
        if c not in ts:
            ts[c] = dict(
                t_all=pc.tile([P, NCHUNK, D], F32, name=f"tall{c}", tag="tall"),
                bst=pc.tile([P, NCHUNK, 6], F32, name=f"bst{c}", tag="bst"),
                mv=pc.tile([P, NCHUNK, 2], F32, name=f"mv{c}", tag="mv"),
            )
        t = ts[c]
        fc_ps = wtile([P, 4 * D], F32, f"fc{c}{g}")
        for j in range(4):
            i = 4 * g + j
            nc.tensor.matmul(fc_ps[:, j * D:(j + 1) * D],
                             lhsT=s["ctxT"][:, i * P:(i + 1) * P],
                             rhs=wfc, start=True, stop=True)
        for j in range(4):
            i = 4 * g + j
            nc.vector.scalar_tensor_tensor(
                out=t["t_all"][:, i, :], in0=fc_ps[:, j * D:(j + 1) * D],
                scalar=1.0, in1=s["xq_nat"][:, i, :].bitcast(F32),
                op0=A.mult, op1=A.add)
            nc.vector.bn_stats(out=t["bst"][:, i, :], in_=t["t_all"][:, i, :])

    def ln_stats(c, lo=0, hi=NCHUNK):
        """per-chunk bn_aggr -> (mean, var); rstd via bit-hack + 2 Newton
        iterations (DVE)"""
        t = ts[c]
        if "var" not in t:
            t["var"] = pc.tile([P, NCHUNK], F32, name=f"var{c}", tag="var")
            t["y"] = pc.tile([P, NCHUNK], F32, name=f"y{c}", tag="y")
            t["t1"] = pc.tile([P, NCHUNK], F32, name=f"t1n{c}", tag="t1n")
        sl = slice(lo, hi)
        var, y, t1 = t["var"][:, sl], t["y"][:, sl], t["t1"][:, sl]
        for i in range(lo, hi):
            nc.vector.bn_aggr(out=t["mv"][:, i, :], in_=t["bst"][:, i, :])
        nc.vector.tensor_scalar_add(
            out=var, in0=t["mv"][:, sl, 1:2].rearrange("p i o -> p (i o)"),
            scalar1=EPS)
        nc.vector.tensor_scalar(
            out=y.bitcast(I32), in0=var.bitcast(I32), scalar1=1,
            scalar2=None, op0=A.logical_shift_right)
        nc.vector.tensor_scalar(
            out=y.bitcast(I32), in0=y.bitcast(I32), scalar1=-1,
            scalar2=None, op0=A.bitwise_xor)
        nc.vector.tensor_scalar(
            out=y.bitcast(I32), in0=y.bitcast(I32), scalar1=0x5F3759E0,
            scalar2=None, op0=A.add)
        for _ in range(2):
            nc.vector.tensor_mul(out=t1, in0=y, in1=y)
            nc.vector.tensor_mul(out=t1, in0=t1, in1=var)
            nc.vector.tensor_scalar(out=t1, in0=t1, scalar1=-0.5, scalar2=1.5,
                                    op0=A.mult, op1=A.add)
            nc.vector.tensor_mul(out=y, in0=y, in1=t1)

    def ln_out(c, lo, hi, on_act=False):
        """(t - mean) * rstd, chunks lo..hi-1.  on_act runs it on the
        Scalar engine as t*rstd + (-mean*rstd) - only useful in the
        epilogue when the exps are finished and ACT is idle."""
        t = ts[c]
        if "out_sb" not in t:
            t["out_sb"] = pc.tile([P, NCHUNK, D], F32, name=f"osb{c}",
                                  tag="osb")
        if on_act:
            if "nmb" not in t:
                t["nmb"] = pc.tile([P, NCHUNK], F32, name=f"nmb{c}",
                                   tag="nmb")
            nc.vector.scalar_tensor_tensor(
                out=t["nmb"][:, lo:hi],
                in0=t["mv"][:, lo:hi, 0:1].rearrange("p i o -> p (i o)"),
                scalar=-1.0, in1=t["y"][:, lo:hi], op0=A.mult, op1=A.mult)
            for i in range(lo, hi):
                nc.scalar.activation(
                    out=t["out_sb"][:, i, :], in_=t["t_all"][:, i, :],
                    func=mybir.ActivationFunctionType.Identity,
                    scale=t["y"][:, i:i + 1], bias=t["nmb"][:, i:i + 1])
            return
        for i in range(lo, hi):
            nc.vector.tensor_scalar(
                out=t["out_sb"][:, i, :], in0=t["t_all"][:, i, :],
                scalar1=t["mv"][:, i, 0:1], scalar2=t["y"][:, i:i + 1],
                op0=A.subtract, op1=A.mult)
        if apply_affine:
            for i in range(lo, hi):
                nc.vector.tensor_mul(out=t["out_sb"][:, i, :],
                                     in0=t["out_sb"][:, i, :], in1=gam_tile)
                nc.vector.tensor_add(out=t["out_sb"][:, i, :],
                                     in0=t["out_sb"][:, i, :], in1=bet_tile)

    def store(c, half=None):
        t = ts[c]
        out_r = out_d[c].rearrange("(i p) d -> p i d", p=P)
        if half is None:
            nc.sync.dma_start(out=out_r, in_=t["out_sb"])
        elif half == 0:
            nc.sync.dma_start(out=out_r[:, 0:4, :], in_=t["out_sb"][:, 0:4, :])
        else:
            nc.sync.dma_start(out=out_r[:, 4:8, :], in_=t["out_sb"][:, 4:8, :])

    # ------------- channel-0 minimal critical path to first exp -------------
    alloc_chunk_state(0)
    tr(0, "q", 0)
    tr(0, "q", 1)
    proj_qk(0, "q", 0, on_act=True)
    proj_qk(0, "q", 1)
    tr(0, "k", 0)
    proj_qk(0, "k", 0, on_act=True)
    wpad()
    scores_exp(0, 0)

    # per-slot fillers; emitted AFTER the slot's scores+ctx matmuls.
    def fillers(c, i):
        if c == 0:
            if i == 0:
                tr(0, "k", 1)
                proj_qk(0, "k", 1)
                tr(0, "v", 0)
                v_g(0, 0)
            elif i == 1:
                tr(0, "v", 1)
                v_g(0, 1)
                loads(1)
            elif i == 2:
                tr(1, "q")
            elif i == 3:
                tr(1, "k")
            elif i == 4:
                tr(1, "v")
            elif i == 5:
                proj_qk(1, "q", 0)
            elif i == 6:
                proj_qk(1, "q", 1)
                proj_qk(1, "k", 0)
                proj_qk(1, "k", 1)
            # i == 7 left empty so scores(1, 0) is never gated on a filler
            # cast stuck behind slot-7 DVE work
        else:
            # lag-2 ctx pipeline: ctx(c-1, 7) lands in slot (c, 1), so the
            # previous channel's tail shifts one slot later
            if i == 0:
                v_g(c, 0)
                v_g(c, 1)
                if c + 1 < C:
                    loads(c + 1)
            elif i == 1:
                tail_a(c - 1)
            elif i == 2:
                if c + 1 < C:
                    tr(c + 1, "q")
                fc_group(c - 1, 0)
            elif i == 3:
                if c + 1 < C:
                    tr(c + 1, "k")
                fc_group(c - 1, 1)
            elif i == 4:
                if c + 1 < C:
                    tr(c + 1, "v")
                ln_stats(c - 1)
            elif i == 5:
                if c + 1 < C:
                    proj_qk(c + 1, "q", 0)
                ln_out(c - 1, 0, 4)
                store(c - 1, half=0)
            elif i == 6:
                if c + 1 < C:
                    proj_qk(c + 1, "q", 1)
                    proj_qk(c + 1, "k", 0)
                    proj_qk(c + 1, "k", 1)
                ln_out(c - 1, 4, NCHUNK)
                store(c - 1, half=1)
            # i == 7 left empty (see channel-0 note)

    # ---------------- software-pipelined channel loop ----------------
    # lag-2 chunk pipeline (see docstring)
    for c in range(C):
        if c > 0:
            alloc_chunk_state(c)
        st[c]["ctx_ps"] = ctx_psum.tile([D, N], F32, name=f"ctx{c}", tag="ctx")
        for i in range(NCHUNK):
            if not (c == 0 and i == 0):
                scores_exp(c, i)
            pc_, pi = (c, i - 1) if i >= 1 else (c - 1, NCHUNK - 1)
            if pc_ >= 0:
                vsc_mm(pc_, pi, heads=(0, 1))
            if i >= 2:
                ctx_mm(c, i - 2)
            elif c >= 1 and i == 0:
                ctx_mm(c - 1, NCHUNK - 2)
            elif c >= 1 and i == 1:
                ctx_mm(c - 1, NCHUNK - 1)
            fillers(c, i)
            # deferred Schraudolph denominator at slot end (no downstream
            # pressure: its norm_recip feeds ctx a full slot later), then
            # the head-2 normalization of the previous chunk
            flush_reduce()
            if pc_ >= 0:
                vsc_mm(pc_, pi, heads=(2,))
            wpad()

    # ---------------- pipelined epilogue for channel 3 ----------------
    c = C - 1
    s, li = st[c], NCHUNK - 1
    flush_reduce()
    ctx_mm(c, NCHUNK - 2)
    for h in range(H):
        j = li * H + h
        hs = slice(DV * h, DV * (h + 1))
        nc.gpsimd.normalize_recip(
            out_ap=s["vsc_all"][:, j, :], in_ap=s["v_nat"][:, li, hs],
            denom_ap=s["ssum"][:, j:j + 1])
        for g in range(2):
            nc.tensor.matmul(
                s["ctx_ps"][hs, g * QT:(g + 1) * QT],
                lhsT=s["vsc_all"][:, j, :],
                rhs=s["e_all"][:, j, g * QT:(g + 1) * QT],
                start=False, stop=True, skip_group_check=True)
    tail_a(c, half=0)
    fc_group(c, 0)
    tail_a(c, half=1)
    ln_stats(c, 0, 4)
    fc_group(c, 1)
    out_r = out_d[c].rearrange("(i p) d -> p i d", p=P)
    t3 = None
    for lo in range(0, NCHUNK, 2):
        if lo == 4:
            ln_stats(c, 4, NCHUNK)
        ln_out(c, lo, lo + 2, on_act=True)
        t3 = ts[c]
        nc.sync.dma_start(out=out_r[:, lo:lo + 2, :],
                          in_=t3["out_sb"][:, lo:lo + 2, :])


def _build(apply_affine):
    nc = bacc.Bacc("TRN2", target_bir_lowering=False, debug=False, num_devices=B)
    with tile.TileContext(nc) as tc, ExitStack() as ctx:
        _emit(nc, tc, ctx, apply_affine)
    nc.compile()
    return nc


def _pack_in_maps(input_Q, input_K, input_V, W_Q, W_K, W_V, W_fc,
                  ln_gamma, ln_beta):
    bf = ml_dtypes.bfloat16
    wall = np.zeros((P, D + 2), dtype=np.float32)
    wall[0:D, 0:D] = W_fc.T
    wall[0:D, D] = ln_gamma
    wall[0:D, D + 1] = ln_beta
    wallb = np.zeros((P, P + 3 * D), dtype=bf)
    wallb[:, :P] = np.eye(P, dtype=np.float32).astype(bf)
    for k, W in enumerate((W_Q, W_K, W_V)):
        wallb[0:D, P + k * D:P + (k + 1) * D] = W.T.astype(bf)
    xqb = input_Q.astype(bf)
    xkb = input_K.astype(bf)
    xvb = input_V.astype(bf)
    return [
        {"xq": input_Q[b], "xqb": xqb[b], "xkb": xkb[b], "xvb": xvb[b],
         "wall": wall, "wallb": wallb}
        for b in range(B)
    ]


def kernel(input_Q, input_K, input_V, W_Q, W_K, W_V, W_fc, ln_gamma, ln_beta):
    input_Q = np.ascontiguousarray(np.asarray(input_Q, dtype=np.float32))
    input_K = np.ascontiguousarray(np.asarray(input_K, dtype=np.float32))
    input_V = np.ascontiguousarray(np.asarray(input_V, dtype=np.float32))
    W_Q = np.ascontiguousarray(np.asarray(W_Q, dtype=np.float32))
    W_K = np.ascontiguousarray(np.asarray(W_K, dtype=np.float32))
    W_V = np.ascontiguousarray(np.asarray(W_V, dtype=np.float32))
    W_fc = np.ascontiguousarray(np.asarray(W_fc, dtype=np.float32))
    ln_gamma = np.ascontiguousarray(np.asarray(ln_gamma, dtype=np.float32))
    ln_beta = np.ascontiguousarray(np.asarray(ln_beta, dtype=np.float32))

    apply_affine = not (np.all(ln_gamma == 1.0) and np.all(ln_beta == 0.0))

    key = ("nc", apply_affine)
    if key not in _CACHE:
        _CACHE[key] = _build(apply_affine)
    nc = _CACHE[key]

    in_maps = _pack_in_maps(input_Q, input_K, input_V, W_Q, W_K, W_V, W_fc,
                            ln_gamma, ln_beta)
    res = run_bass_kernel_spmd(nc, in_maps, core_ids=list(range(B)))
    return np.stack([res.results[b]["out"] for b in range(B)], axis=0)


# revision 29
# speedup vs baseline: 1.0746x; 1.0331x over previous
"""Trainium2 Bass kernel for a multi-head-attention block (B,C,N,D = 8,4,1024,96;
H=3 heads, dk=dv=32; softmax over the QUERY axis; residual + LayerNorm).

Sharding: pure data-parallel over batch B across 8 NeuronCores (one batch
element per core, C=4 channel-slices each, no collectives).

Schedule notes (v2):
  - exp work is split across engines: per chunk, heads 0/1 exp on ACT
    (spline exp + fused accumulator denominator + accumulator read), and
    head 2 of DVE-light slots runs a Schraudolph bit-hack exp on the DVE
    (one tensor_scalar: int16 = round(scores*A + B), bitcast bf16) with a
    deferred tensor_reduce denominator.  Per-tile constant factors cancel
    exactly in the q-axis softmax, so the ~4% pointwise exp error averages
    to ~1e-3 relative in the final output.
  - softmax normalization (vsc = V/denom, plus 1/denom write-back nobody
    reads) runs on the otherwise-idle GPSIMD via normalize_recip (attn
    ucode library), freeing the DVE.
  - PSUM pools are split so the scores ring (w_psum, 2 bufs x 2 banks)
    holds ONLY score tiles: the ring dependency is exp(i,h) -> scores MM,
    an ACT->PE edge with no DVE cast interposed.  Filler tiles
    (transpose/projection/v/fc) live in their own 2x1-bank pool (fp) and
    self-pace against their casts.  ctx accumulator: 2 banks.  8 total.
  - inputs arrive in DRAM twice: f32 xq (residual path) and host-rounded
    bf16 xq/xk/xv (projection path).  PE transposes are is_transpose
    matmuls with a bf16 identity writing bf16 PSUM (half the banks, one
    2x-mode cast per input instead of two 1x f32 casts).
  - lag-2 chunk pipeline: norm_recip(c,i-1) behind scores/exp(c,i), and
    ctx(c,i-2) one slot later so the PE never waits the gpsimd round trip;
    channel boundaries keep the same cadence.
  - LN tail: residual via scalar_tensor_tensor, bn_stats/bn_aggr for
    mean/var, rsqrt via bit-hack + 2 Newton iterations (all DVE);
    channel-3 tail is pipelined per q-half with ACT-based ln_out.
"""

from contextlib import ExitStack

import ml_dtypes
import numpy as np

import concourse.bass as bass
import concourse.tile as tile
from concourse import bacc, mybir
from concourse.bass_utils import run_bass_kernel_spmd

F32 = mybir.dt.float32
BF16 = mybir.dt.bfloat16
F32R = mybir.dt.float32r
I32 = mybir.dt.int32
I16 = mybir.dt.int16
A = mybir.AluOpType

B, C, N, D = 8, 4, 1024, 96
H, DK, DV = 3, 32, 32
P = 128               # partition size / token chunk
NCHUNK = N // P       # 8
QT = 512              # matmul free-dim limit into one PSUM bank (f32)
HN = N // 2
SCALE = 1.0 / np.sqrt(DK)
EPS = 1e-5

# Schraudolph exp on the DVE: bits(bf16 e) = round(x*SCALE*A + B) as int16.
SCH_A = float((2.0 ** 7) / np.log(2.0) * SCALE)
SCH_B = float(127 * 2 ** 7)


def _dve_tile(c, i):
    """Chunks whose head-2 exp runs on the DVE instead of ACT: the
    engine-balance knob, placed in DVE-light slots only.  The final chunk
    stays on ACT so the epilogue's normalization chain starts sooner."""
    if c == C - 1 and i == NCHUNK - 1:
        return False
    return (c >= 1 and i in (0, 1, 5, 7)) or (c == 0 and i in (1, 7))

_CACHE = {}


def _emit(nc, tc, ctx, apply_affine):
    xq_d = nc.dram_tensor("xq", [C, N, D], F32R, kind="ExternalInput").ap()
    xqb_d = nc.dram_tensor("xqb", [C, N, D], BF16, kind="ExternalInput").ap()
    xkb_d = nc.dram_tensor("xkb", [C, N, D], BF16, kind="ExternalInput").ap()
    xvb_d = nc.dram_tensor("xvb", [C, N, D], BF16, kind="ExternalInput").ap()
    # wall  = host-packed f32 [128, 96+2]: wfc.T (zero-padded) | gamma | beta
    # wallb = host-packed bf16 [128, 128 + 3*96]: identity | wq|wk|wv (each
    #         W.T, [96,96] natural, zero-padded to 128 rows)
    wall_d = nc.dram_tensor("wall", [P, D + 2], F32R,
                            kind="ExternalInput").ap()
    wallb_d = nc.dram_tensor("wallb", [P, P + 3 * D], BF16,
                             kind="ExternalInput").ap()
    out_d = nc.dram_tensor("out", [C, N, D], F32, kind="ExternalOutput").ap()

    const = ctx.enter_context(tc.tile_pool(name="const", bufs=1))
    pc = ctx.enter_context(tc.tile_pool(name="perc", bufs=2))
    w_psum = ctx.enter_context(tc.tile_pool(name="w_psum", bufs=3, space="PSUM"))
    ctx_psum = ctx.enter_context(tc.tile_pool(name="ctx_psum", bufs=1, space="PSUM"))

    # ---- w_psum ring discipline: pad allocations to multiples of 3 so the
    # 3 scores tiles of chunk i+1 land exactly on the banks freed by the
    # 3 exps of chunk i (same head -> earliest possible reuse).
    wct = {"n": 0, "pad": 0}

    def wtile(shape, dtype, name):
        wct["n"] += 1
        return w_psum.tile(shape, dtype, name=name, tag="w")

    def wpad():
        while wct["n"] % 3:
            wct["n"] += 1
            wct["pad"] += 1
            w_psum.tile([P, 8], F32, name=f"pad{wct['pad']}", tag="w")

    # ---------------- prologue: DMAs + PE warm-up spins ----------------
    dummy = const.tile([P, QT], BF16)
    nc.vector.memset(dummy, 0)

    # identity (gates the first transposes) before the weight columns
    wallb = const.tile([P, P + 3 * D], BF16)
    nc.sync.dma_start(out=wallb[:, 0:P], in_=wallb_d[:, 0:P])
    nc.sync.dma_start(out=wallb[:, P:], in_=wallb_d[:, P:])
    ident_b = wallb[:, 0:P]
    wb = {nm: wallb[0:D, P + k * D:P + (k + 1) * D]
          for k, nm in enumerate(("wq", "wk", "wv"))}

    # f32 side: wfc (first needed at fc_group(0) in slot (1,2)) + ln affine
    wall = const.tile([P, D + 2], F32R)
    nc.gpsimd.dma_start(out=wall, in_=wall_d)
    wfc = wall[0:D, 0:D]

    # channel-0 loads: bf16 projections path split for earliest q
    xq0 = pc.tile([P, NCHUNK, D], F32R, name="xq_nat0", tag="xq_nat", bufs=3)
    xqb0 = pc.tile([P, NCHUNK, D], BF16, name="xqb0", tag="xqb", bufs=1)
    xkb0 = pc.tile([P, NCHUNK, D], BF16, name="xkb0", tag="xkb", bufs=1)
    xvb0 = pc.tile([P, NCHUNK, D], BF16, name="xvb0", tag="xvb", bufs=1)
    xqb0_r = xqb_d[0].rearrange("(i p) d -> p i d", p=P)
    xkb0_r = xkb_d[0].rearrange("(i p) d -> p i d", p=P)
    # ACT's DMA queue is free before the first exp; one DMA per queue to
    # avoid per-queue DGE serialization
    nc.scalar.dma_start(out=xqb0[:, 0:4, :], in_=xqb0_r[:, 0:4, :])
    nc.gpsimd.dma_start(out=xqb0[:, 4:8, :], in_=xqb0_r[:, 4:8, :])
    nc.gpsimd.dma_start(out=xkb0[:, 0:4, :], in_=xkb0_r[:, 0:4, :])
    nc.gpsimd.dma_start(out=xkb0[:, 4:8, :], in_=xkb0_r[:, 4:8, :])
    nc.gpsimd.dma_start(out=xvb0, in_=xvb_d[0].rearrange("(i p) d -> p i d", p=P))
    nc.sync.dma_start(out=xq0, in_=xq_d[0].rearrange("(i p) d -> p i d", p=P))

    spin = wtile([P, N], F32, "spin")
    for _ in range(2):
        nc.tensor.matmul(spin[0:64, 0:256], lhsT=dummy[:, 0:64],
                         rhs=dummy[:, 0:256], start=True, stop=True,
                         skip_group_check=True)

    # warm up the gpsimd attn ucode library (normalize_recip): the inserted
    # MODIFY_POOL_CONFIG + ~6us IRAM load runs now, hidden under channel-0
    # compute, instead of stalling the first real norm_recip mid-pipeline
    warm = const.tile([P, 4], F32)
    nc.vector.memset(warm, 1.0)
    nc.gpsimd.normalize_recip(out_ap=warm[:, 2:3], in_ap=warm[:, 0:1],
                              denom_ap=warm[:, 1:2])

    gam_tile = bet_tile = None
    if apply_affine:
        gam_tile = const.tile([P, D], F32)
        bet_tile = const.tile([P, D], F32)
        for t, col in ((gam_tile, D), (bet_tile, D + 1)):
            col_ap = wall_d[0:D, col:col + 1]
            bcast = bass.AP(tensor=col_ap.tensor, offset=col_ap.offset,
                            ap=[[0, P], col_ap.ap[0]])
            nc.gpsimd.dma_start(out=t, in_=bcast)

    st = {0: dict(xq_nat=xq0, xqb=xqb0, xkb=xkb0, xvb=xvb0, xTs={})}
    ts = {}

    def alloc_chunk_state(c):
        s = st[c]
        s["ssum"] = pc.tile([P, H * NCHUNK], F32, name=f"ssum{c}", tag="ssum")
        s["e_all"] = pc.tile([P, H * NCHUNK, N], BF16, name=f"e{c}", tag="e")
        s["vsc_all"] = pc.tile([P, H * NCHUNK, DV], BF16, name=f"vsc{c}",
                               tag="vsc")

    def loads(c):
        """DMA loads for channel c (c >= 1): no triggers on the Scalar eng."""
        xq = pc.tile([P, NCHUNK, D], F32R, name=f"xq_nat{c}", tag="xq_nat",
                     bufs=3)
        xqb = pc.tile([P, NCHUNK, D], BF16, name=f"xqb{c}", tag="xqb", bufs=1)
        xkb = pc.tile([P, NCHUNK, D], BF16, name=f"xkb{c}", tag="xkb", bufs=1)
        xvb = pc.tile([P, NCHUNK, D], BF16, name=f"xvb{c}", tag="xvb", bufs=1)
        nc.sync.dma_start(out=xq, in_=xq_d[c].rearrange("(i p) d -> p i d", p=P))
        nc.sync.dma_start(out=xqb, in_=xqb_d[c].rearrange("(i p) d -> p i d", p=P))
        nc.gpsimd.dma_start(out=xkb, in_=xkb_d[c].rearrange("(i p) d -> p i d", p=P))
        nc.gpsimd.dma_start(out=xvb, in_=xvb_d[c].rearrange("(i p) d -> p i d", p=P))
        st[c] = dict(xq_nat=xq, xqb=xqb, xkb=xkb, xvb=xvb, xTs={})

    def tr(c, nm, g=None):
        """PE is_transpose (bf16 in/out PSUM) of input nm; g=None does all
        8 chunks with a single 2x-mode cast, g in {0,1} does half."""
        s = st[c]
        src = s[f"x{nm}b"]
        if nm not in s["xTs"]:
            s["xTs"][nm] = pc.tile([D, N], BF16, name=f"x{nm}T{c}",
                                   tag=f"x{nm}T", bufs=1)
        xT = s["xTs"][nm]
        if g is None:
            tp = wtile([D, N], BF16, f"tp{nm}{c}")
            for i in range(NCHUNK):
                nc.tensor.transpose(tp[:, i * P:(i + 1) * P],
                                    in_=src[:, i, :], identity=ident_b)
            nc.vector.tensor_copy(out=xT, in_=tp)
        else:
            tp = wtile([D, HN], BF16, f"tp{nm}{c}{g}")
            for j in range(4):
                i = 4 * g + j
                nc.tensor.transpose(tp[:, j * P:(j + 1) * P],
                                    in_=src[:, i, :], identity=ident_b)
            nc.vector.tensor_copy(out=xT[:, g * HN:(g + 1) * HN], in_=tp)

    def proj_qk(c, which, g, on_act=False):
        """Q or K projection into [e, tok] bf16 layout, qtile g.  on_act
        moves the PSUM->SBUF cast to the (idle) Scalar engine -- prologue
        only, to parallelize the cast chain to the first exp."""
        s = st[c]
        nm, w_t = (("qdT", wb["wq"]) if which == "q" else ("kdT", wb["wk"]))
        if nm not in s:
            s[nm] = pc.tile([D, N], BF16, name=f"{nm}{c}", tag=nm)
        dst, xT = s[nm], s["xTs"][which]
        pr_ps = wtile([D, QT], F32, f"pr{c}{which}{g}")
        nc.tensor.matmul(pr_ps, lhsT=w_t, rhs=xT[:, g * QT:(g + 1) * QT],
                         start=True, stop=True)
        if on_act:
            nc.scalar.copy(out=dst[:, g * QT:(g + 1) * QT], in_=pr_ps)
        else:
            nc.vector.tensor_copy(out=dst[:, g * QT:(g + 1) * QT], in_=pr_ps)

    def v_g(c, g):
        """V projection (natural f32 layout), chunks 4g..4g+3"""
        s = st[c]
        if "v_nat" not in s:
            s["v_nat"] = pc.tile([P, NCHUNK, D], F32, name=f"v_nat{c}",
                                 tag="v_nat")
        v_nat = s["v_nat"]
        v_ps = wtile([P, 4 * D], F32, f"vps{c}{g}")
        for j in range(4):
            i = 4 * g + j
            nc.tensor.matmul(v_ps[:, j * D:(j + 1) * D],
                             lhsT=s["xTs"]["v"][:, i * P:(i + 1) * P],
                             rhs=wb["wv"], start=True, stop=True)
        nc.vector.tensor_copy(
            out=v_nat[:, 4 * g:4 * (g + 1), :].rearrange("p i d -> p (i d)"),
            in_=v_ps)

    pending_reduce = []

    def exp_tile(c, i, h):
        """exp for one (chunk, head) score tile.  ACT: spline exp + fused
        accumulator.  DVE: Schraudolph tensor_scalar; its tensor_reduce
        denominator is deferred to the slot end (a full slot of slack
        before the gpsimd norm_recip needs it)."""
        s = st[c]
        j = i * H + h
        if h == 2 and _dve_tile(c, i):
            nc.vector.tensor_scalar(
                out=s["e_all"][:, j, :].bitcast(I16), in0=s["s_regs"][h],
                scalar1=SCH_A, scalar2=SCH_B, op0=A.mult, op1=A.add)
            pending_reduce.append((c, j))
        else:
            nc.scalar.activation(
                out=s["e_all"][:, j, :], in_=s["s_regs"][h],
                func=mybir.ActivationFunctionType.Exp,
                scale=SCALE, accum_out=s["ssum"][:, j:j + 1])

    def flush_reduce():
        while pending_reduce:
            c_, j_ = pending_reduce.pop(0)
            s_ = st[c_]
            nc.vector.tensor_reduce(
                out=s_["ssum"][:, j_:j_ + 1], in_=s_["e_all"][:, j_, :],
                axis=mybir.AxisListType.X, op=A.add)

    def scores_exp(c, i):
        """S_T + exp for chunk i.  The three heads' matmuls are adjacent at
        row-groups 0/32/64 so they run concurrently in the PE array."""
        s = st[c]
        s_regs = [wtile([P, N], F32, f"s{c}_{i}_{h}") for h in range(H)]
        s["s_regs"] = s_regs
        # head-major, head 2 last: its PSUM ring slot frees on the previous
        # chunk's DVE Schraudolph, which must not delay heads 0/1 (the ACT
        # critical path)
        for h in (0, 1, 2):
            for g in range(2):
                hs = slice(DK * h, DK * (h + 1))
                nc.tensor.matmul(
                    s_regs[h][:, g * QT:(g + 1) * QT],
                    lhsT=s["kdT"][hs, i * P:(i + 1) * P],
                    rhs=s["qdT"][hs, g * QT:(g + 1) * QT],
                    start=True, stop=True)
        for h in range(H):
            exp_tile(c, i, h)

    def vsc_one(c, i, h):
        s = st[c]
        hs = slice(DV * h, DV * (h + 1))
        j = i * H + h
        nc.gpsimd.normalize_recip(
            out_ap=s["vsc_all"][:, j, :], in_ap=s["v_nat"][:, i, hs],
            denom_ap=s["ssum"][:, j:j + 1])

    def vsc_mm(c, i, heads=range(H)):
        """fold 1/denom into V rows for chunk i: gpsimd normalize_recip
        (vsc = v/denom; ssum overwritten with 1/denom, unread)."""
        for h in heads:
            vsc_one(c, i, h)

    def ctx_mm(c, i):
        """context accumulation for chunk i: bf16, three heads at
        col-groups 0/32/64, emitted adjacently -> concurrent."""
        s = st[c]
        # head-major with head 2 last: its vsc lands latest (end of the
        # previous slot), so its matmuls must not head-of-line-block the
        # other heads' ctx or anything behind them in the PE queue
        for h in (0, 1, 2):
            for g in range(2):
                hs = slice(DV * h, DV * (h + 1))
                j = i * H + h
                nc.tensor.matmul(
                    s["ctx_ps"][hs, g * QT:(g + 1) * QT],
                    lhsT=s["vsc_all"][:, j, :],
                    rhs=s["e_all"][:, j, g * QT:(g + 1) * QT],
                    start=(i == 0), stop=(i == NCHUNK - 1),
                    skip_group_check=True)

    def tail_a(c, half=None):
        """ctx copy-out (frees the ctx PSUM banks); half in {0,1,None}"""
        s = st[c]
        if "ctxT" not in s:
            s["ctxT"] = pc.tile([D, N], F32R, name=f"ctxT{c}", tag="ctxT")
        halves = (0, 1) if half is None else (half,)
        for g in halves:
            nc.vector.tensor_copy(out=s["ctxT"][:, g * QT:(g + 1) * QT],
                                  in_=s["ctx_ps"][:, g * QT:(g + 1) * QT])
        if half in (1, None):
            del s["ctx_ps"]

    def fc_group(c, g):
        """fc matmuls + residual add + one bn_stats pass for chunks 4g..4g+3"""
        s = st[c]
        if c not in ts:
            ts[c] = dict(
                t_all=pc.tile([P, NCHUNK, D], F32, name=f"tall{c}", tag="tall"),
                bst=pc.tile([P, NCHUNK, 6], F32, name=f"bst{c}", tag="bst"),
                mv=pc.tile([P, NCHUNK, 2], F32, name=f"mv{c}", tag="mv"),
            )
        t = ts[c]
        fc_ps = wtile([P, 4 * D], F32, f"fc{c}{g}")
        for j in range(4):
            i = 4 * g + j
            nc.tensor.matmul(fc_ps[:, j * D:(j + 1) * D],
                             lhsT=s["ctxT"][:, i * P:(i + 1) * P],
                             rhs=wfc, start=True, stop=True)
        for j in range(4):
            i = 4 * g + j
            nc.vector.scalar_tensor_tensor(
                out=t["t_all"][:, i, :], in0=fc_ps[:, j * D:(j + 1) * D],
                scalar=1.0, in1=s["xq_nat"][:, i, :].bitcast(F32),
                op0=A.mult, op1=A.add)
            nc.vector.bn_stats(out=t["bst"][:, i, :], in_=t["t_all"][:, i, :])

    def ln_stats(c, lo=0, hi=NCHUNK):
        """per-chunk bn_aggr -> (mean, var); rstd via bit-hack + 2 Newton
        iterations (DVE)"""
        t = ts[c]
        if "var" not in t:
            t["var"] = pc.tile([P, NCHUNK], F32, name=f"var{c}", tag="var")
            t["y"] = pc.tile([P, NCHUNK], F32, name=f"y{c}", tag="y")
            t["t1"] = pc.tile([P, NCHUNK], F32, name=f"t1n{c}", tag="t1n")
        sl = slice(lo, hi)
        var, y, t1 = t["var"][:, sl], t["y"][:, sl], t["t1"][:, sl]
        for i in range(lo, hi):
            nc.vector.bn_aggr(out=t["mv"][:, i, :], in_=t["bst"][:, i, :])
        nc.vector.tensor_scalar_add(
            out=var, in0=t["mv"][:, sl, 1:2].rearrange("p i o -> p (i o)"),
            scalar1=EPS)
        nc.vector.tensor_scalar(
            out=y.bitcast(I32), in0=var.bitcast(I32), scalar1=1,
            scalar2=None, op0=A.logical_shift_right)
        nc.vector.tensor_scalar(
            out=y.bitcast(I32), in0=y.bitcast(I32), scalar1=-1,
            scalar2=None, op0=A.bitwise_xor)
        nc.vector.tensor_scalar(
            out=y.bitcast(I32), in0=y.bitcast(I32), scalar1=0x5F3759E0,
            scalar2=None, op0=A.add)
        for _ in range(2):
            nc.vector.tensor_mul(out=t1, in0=y, in1=y)
            nc.vector.tensor_mul(out=t1, in0=t1, in1=var)
            nc.vector.tensor_scalar(out=t1, in0=t1, scalar1=-0.5, scalar2=1.5,
                                    op0=A.mult, op1=A.add)
            nc.vector.tensor_mul(out=y, in0=y, in1=t1)

    def ln_out(c, lo, hi, on_act=False):
        """(t - mean) * rstd, chunks lo..hi-1.  on_act runs it on the
        Scalar engine as t*rstd + (-mean*rstd) - only useful in the
        epilogue when the exps are finished and ACT is idle."""
        t = ts[c]
        if "out_sb" not in t:
            t["out_sb"] = pc.tile([P, NCHUNK, D], F32, name=f"osb{c}",
                                  tag="osb")
        if on_act:
            if "nmb" not in t:
                t["nmb"] = pc.tile([P, NCHUNK], F32, name=f"nmb{c}",
                                   tag="nmb")
            nc.vector.scalar_tensor_tensor(
                out=t["nmb"][:, lo:hi],
                in0=t["mv"][:, lo:hi, 0:1].rearrange("p i o -> p (i o)"),
                scalar=-1.0, in1=t["y"][:, lo:hi], op0=A.mult, op1=A.mult)
            for i in range(lo, hi):
                nc.scalar.activation(
                    out=t["out_sb"][:, i, :], in_=t["t_all"][:, i, :],
                    func=mybir.ActivationFunctionType.Identity,
                    scale=t["y"][:, i:i + 1], bias=t["nmb"][:, i:i + 1])
            return
        for i in range(lo, hi):
            nc.vector.tensor_scalar(
                out=t["out_sb"][:, i, :], in0=t["t_all"][:, i, :],
                scalar1=t["mv"][:, i, 0:1], scalar2=t["y"][:, i:i + 1],
                op0=A.subtract, op1=A.mult)
        if apply_affine:
            for i in range(lo, hi):
                nc.vector.tensor_mul(out=t["out_sb"][:, i, :],
                                     in0=t["out_sb"][:, i, :], in1=gam_tile)
                nc.vector.tensor_add(out=t["out_sb"][:, i, :],
                                     in0=t["out_sb"][:, i, :], in1=bet_tile)

    def store(c, half=None):
        t = ts[c]
        out_r = out_d[c].rearrange("(i p) d -> p i d", p=P)
        if half is None:
            nc.sync.dma_start(out=out_r, in_=t["out_sb"])
        elif half == 0:
            nc.sync.dma_start(out=out_r[:, 0:4, :], in_=t["out_sb"][:, 0:4, :])
        else:
            nc.sync.dma_start(out=out_r[:, 4:8, :], in_=t["out_sb"][:, 4:8, :])

    # ------------- channel-0 minimal critical path to first exp -------------
    alloc_chunk_state(0)
    tr(0, "q", 0)
    tr(0, "q", 1)
    proj_qk(0, "q", 0, on_act=True)
    proj_qk(0, "q", 1)
    tr(0, "k", 0)
    proj_qk(0, "k", 0, on_act=True)
    wpad()
    scores_exp(0, 0)

    # per-slot fillers; emitted AFTER the slot's scores+ctx matmuls.
    def fillers(c, i):
        if c == 0:
            if i == 0:
                tr(0, "k", 1)
                proj_qk(0, "k", 1)
                tr(0, "v", 0)
                v_g(0, 0)
            elif i == 1:
                tr(0, "v", 1)
                v_g(0, 1)
                loads(1)
            elif i == 2:
                tr(1, "q")
            elif i == 3:
                tr(1, "k")
            elif i == 4:
                tr(1, "v")
            elif i == 5:
                proj_qk(1, "q", 0)
            elif i == 6:
                proj_qk(1, "q", 1)
                proj_qk(1, "k", 0)
                proj_qk(1, "k", 1)
            # i == 7 left empty so scores(1, 0) is never gated on a filler
            # cast stuck behind slot-7 DVE work
        else:
            # lag-2 ctx pipeline: ctx(c-1, 7) lands in slot (c, 1), so the
            # previous channel's tail shifts one slot later
            if i == 0:
                v_g(c, 0)
                v_g(c, 1)
                if c + 1 < C:
                    loads(c + 1)
            elif i == 1:
                tail_a(c - 1)
            elif i == 2:
                if c + 1 < C:
                    tr(c + 1, "q")
                fc_group(c - 1, 0)
            elif i == 3:
                if c + 1 < C:
                    tr(c + 1, "k")
                fc_group(c - 1, 1)
            elif i == 4:
                if c + 1 < C:
                    tr(c + 1, "v")
                ln_stats(c - 1)
            elif i == 5:
                if c + 1 < C:
                    proj_qk(c + 1, "q", 0)
                ln_out(c - 1, 0, 4)
                store(c - 1, half=0)
            elif i == 6:
                if c + 1 < C:
                    proj_qk(c + 1, "q", 1)
                    proj_qk(c + 1, "k", 0)
                    proj_qk(c + 1, "k", 1)
                ln_out(c - 1, 4, NCHUNK)
                store(c - 1, half=1)
            # i == 7 left empty (see channel-0 note)

    # ---------------- software-pipelined channel loop ----------------
    # lag-2 chunk pipeline (see docstring)
    for c in range(C):
        if c > 0:
            alloc_chunk_state(c)
        st[c]["ctx_ps"] = ctx_psum.tile([D, N], F32, name=f"ctx{c}", tag="ctx")
        for i in range(NCHUNK):
            if not (c == 0 and i == 0):
                scores_exp(c, i)
            pc_, pi = (c, i - 1) if i >= 1 else (c - 1, NCHUNK - 1)
            if pc_ >= 0:
                vsc_mm(pc_, pi, heads=(0, 1))
            if i >= 2:
                ctx_mm(c, i - 2)
            elif c >= 1 and i == 0:
                ctx_mm(c - 1, NCHUNK - 2)
            elif c >= 1 and i == 1:
                ctx_mm(c - 1, NCHUNK - 1)
            fillers(c, i)
            # deferred Schraudolph denominator at slot end (no downstream
            # pressure: its norm_recip feeds ctx a full slot later), then
            # the head-2 normalization of the previous chunk
            flush_reduce()
            if pc_ >= 0:
                vsc_mm(pc_, pi, heads=(2,))
            wpad()

    # ---------------- pipelined epilogue for channel 3 ----------------
    c = C - 1
    s, li = st[c], NCHUNK - 1
    flush_reduce()
    ctx_mm(c, NCHUNK - 2)
    for h in range(H):
        j = li * H + h
        hs = slice(DV * h, DV * (h + 1))
        nc.gpsimd.normalize_recip(
            out_ap=s["vsc_all"][:, j, :], in_ap=s["v_nat"][:, li, hs],
            denom_ap=s["ssum"][:, j:j + 1])
        for g in range(2):
            nc.tensor.matmul(
                s["ctx_ps"][hs, g * QT:(g + 1) * QT],
                lhsT=s["vsc_all"][:, j, :],
                rhs=s["e_all"][:, j, g * QT:(g + 1) * QT],
                start=False, stop=True, skip_group_check=True)
    tail_a(c, half=0)
    fc_group(c, 0)
    tail_a(c, half=1)
    ln_stats(c, 0, 4)
    fc_group(c, 1)
    out_r = out_d[c].rearrange("(i p) d -> p i d", p=P)
    t3 = None
    for lo in range(0, NCHUNK, 2):
        if lo == 4:
            ln_stats(c, 4, NCHUNK)
        ln_out(c, lo, lo + 2, on_act=True)
        t3 = ts[c]
        nc.sync.dma_start(out=out_r[:, lo:lo + 2, :],
                          in_=t3["out_sb"][:, lo:lo + 2, :])


def _build(apply_affine):
    nc = bacc.Bacc("TRN2", target_bir_lowering=False, debug=False, num_devices=B)
    with tile.TileContext(nc) as tc, ExitStack() as ctx:
        _emit(nc, tc, ctx, apply_affine)
    nc.compile()
    return nc


def _pack_in_maps(input_Q, input_K, input_V, W_Q, W_K, W_V, W_fc,
                  ln_gamma, ln_beta):
    bf = ml_dtypes.bfloat16
    wall = np.zeros((P, D + 2), dtype=np.float32)
    wall[0:D, 0:D] = W_fc.T
    wall[0:D, D] = ln_gamma
    wall[0:D, D + 1] = ln_beta
    wallb = np.zeros((P, P + 3 * D), dtype=bf)
    wallb[:, :P] = np.eye(P, dtype=np.float32).astype(bf)
    for k, W in enumerate((W_Q, W_K, W_V)):
        wallb[0:D, P + k * D:P + (k + 1) * D] = W.T.astype(bf)
    xqb = input_Q.astype(bf)
    xkb = input_K.astype(bf)
    xvb = input_V.astype(bf)
    return [
        {"xq": input_Q[b], "xqb": xqb[b], "xkb": xkb[b], "xvb": xvb[b],
         "wall": wall, "wallb": wallb}
        for b in range(B)
    ]


def kernel(input_Q, input_K, input_V, W_Q, W_K, W_V, W_fc, ln_gamma, ln_beta):
    input_Q = np.ascontiguousarray(np.asarray(input_Q, dtype=np.float32))
    input_K = np.ascontiguousarray(np.asarray(input_K, dtype=np.float32))
    input_V = np.ascontiguousarray(np.asarray(input_V, dtype=np.float32))
    W_Q = np.ascontiguousarray(np.asarray(W_Q, dtype=np.float32))
    W_K = np.ascontiguousarray(np.asarray(W_K, dtype=np.float32))
    W_V = np.ascontiguousarray(np.asarray(W_V, dtype=np.float32))
    W_fc = np.ascontiguousarray(np.asarray(W_fc, dtype=np.float32))
    ln_gamma = np.ascontiguousarray(np.asarray(ln_gamma, dtype=np.float32))
    ln_beta = np.ascontiguousarray(np.asarray(ln_beta, dtype=np.float32))

    apply_affine = not (np.all(ln_gamma == 1.0) and np.all(ln_beta == 0.0))

    key = ("nc", apply_affine)
    if key not in _CACHE:
        _CACHE[key] = _build(apply_affine)
    nc = _CACHE[key]

    in_maps = _pack_in_maps(input_Q, input_K, input_V, W_Q, W_K, W_V, W_fc,
                            ln_gamma, ln_beta)
    res = run_bass_kernel_spmd(nc, in_maps, core_ids=list(range(B)))
    return np.stack([res.results[b]["out"] for b in range(B)], axis=0)
